# revision 1
# baseline (speedup 1.0000x reference)
"""Trainium2 Bass kernel for the DGNN message-passing module.

Contract: kernel(**inputs) takes the FULL unsharded inputs (see shapes
below) and returns the full [2048, 64] float32 output.  Internally the
leading B (event) dimension is sharded across 8 NeuronCores (pure data
parallel); small weights are replicated.

Math (per core, b=256, H=20, FEAT=HID=128, OUT=64):
  soft1 = softmax(-delta*(e_time[:,None]-his_time), axis=1)
  soft2 = softmax(-delta*(his_time[:,:,None]-his_his_time), axis=2)
  agg1[b]   = sum_h soft1[b,h] * one_hop[b,h,:]          (linearity pull-out)
  agg2[b,h] = sum_k soft2[b,h,k] * two_hop[b,h,k,:]
  x_s_one = relu(self@W0.T + agg1@W2.T + b0+b2)
  x_one_s = relu(one_hop@W0.T + agg2@W2.T + b0+b2)
  y[b]    = sum_h soft1[b,h] * x_one_s[b,h,:]
  out     = x_s_one@W4.T + y@W6.T + b4+b6

The dominant cost is streaming two_hop (50 MB/core).  The weighted
segment-sum agg2 runs on the tensor engine: for each 128-row tile of
two_hop (lhsT, natural layout) we matmul against a [128, <=8] "block
diagonal" tile = const 0/1 mask * exp(logit) per-partition column, and
accumulate group columns in PSUM.  Softmax normalization is folded into
the PSUM eviction (multiply by replicated 1/Z).
"""

import sys

import numpy as np

sys.path.insert(0, "/opt/trn_rl_repo")

B, HIST, FEAT, HID, OUT = 2048, 20, 128, 128, 64
NCORES = 8
BC = B // NCORES          # 256 events per core
G = BC * HIST             # 5120 (b,h) groups per core
R2 = G * HIST             # 102400 two-hop rows per core
ST_COLS = 512             # PSUM group-columns per supertile (1 bank of fp32)

# (128*t) % 20 cycles with period 5; per-phase mask width (# groups touched
# by a 128-row pass).
PHIS = [0, 8, 16, 4, 12]


def _phase_width(phi: int) -> int:
    return (phi + 127) // 20 + 1


def build_bdmask() -> np.ndarray:
    """[128, 40] = 5 masks of [128, 8]: mask[p, 8*i + m] = 1 if (phi_i+p)//20 == m."""
    m = np.zeros((128, 40), np.float32)
    for i, phi in enumerate(PHIS):
        for p in range(128):
            m[p, 8 * i + (phi + p) // 20] = 1.0
    return m


def build_program(bc: int = BC, repeat: int = 1, mode: str = "full"):
    """Build the SPMD Bass program (one NeuronCore's view). Returns nc.

    repeat>1 duplicates the whole compute body (timing harness only).
    mode: "full" | "dmaonly" (stream two_hop, skip phase-1 compute) |
    "nodma" (skip the two_hop stream DMAs)."""
    import concourse.bass as bass
    import concourse.tile as tile
    from concourse import bacc, mybir
    from contextlib import ExitStack

    F32 = mybir.dt.float32
    AF = mybir.ActivationFunctionType
    g = bc * HIST
    r2 = g * HIST
    nbt = bc // 128              # b-chunks (2)
    nt1 = g // 128               # 128-row passes over one_hop / x_one_s (40)
    nst = (g + ST_COLS - 1) // ST_COLS

    nc = bacc.Bacc("TRN2", target_bir_lowering=False, debug=False)

    def din(name, shape):
        return nc.dram_tensor(name, list(shape), F32, kind="ExternalInput").ap()

    two_hop = din("two_hop", (r2, FEAT))
    one_hop = din("one_hop", (g, FEAT))
    one_hop_t = din("one_hop_t", (FEAT, g))
    self_t = din("self_t", (FEAT, bc))
    l1 = din("l1", (bc, HIST))            # delta*(his_time - e_time[:,None])
    l2n = din("l2n", (bc, HIST * HIST))   # delta*(his_his - his_time[:,:,None])
    l2f = din("l2f", (128, r2 // 128))    # same, flat-transposed [p, t] = v[128t+p]
    w0t = din("w0t", (FEAT, HID))
    w2t = din("w2t", (FEAT, HID))
    w4t = din("w4t", (HID, OUT))
    w6t = din("w6t", (HID, OUT))
    b01 = din("b01", (1, HID))
    b46 = din("b46", (1, OUT))
    bdmask = din("bdmask", (128, 40))
    ident = din("ident", (128, 128))
    out_d = nc.dram_tensor("out", [bc, OUT], F32, kind="ExternalOutput").ap()

    with tile.TileContext(nc) as tc, ExitStack() as ctx:
        const = ctx.enter_context(tc.tile_pool(name="const", bufs=1))
        sbig = ctx.enter_context(tc.tile_pool(name="sbig", bufs=1))
        xpool = ctx.enter_context(tc.tile_pool(name="xp", bufs=8))
        bdpool = ctx.enter_context(tc.tile_pool(name="bdp", bufs=4))
        spool = ctx.enter_context(tc.tile_pool(name="sp", bufs=4))
        dpool = ctx.enter_context(tc.tile_pool(name="dram", bufs=1, space="DRAM"))
        p_agg = ctx.enter_context(tc.tile_pool(name="pagg", bufs=2, space="PSUM"))
        p_misc = ctx.enter_context(tc.tile_pool(name="pmisc", bufs=2, space="PSUM"))
        p_acc = ctx.enter_context(tc.tile_pool(name="pacc", bufs=1, space="PSUM"))

        def cload(ap, shape, tag):
            t = const.tile(list(shape), F32, tag=tag)
            nc.sync.dma_start(t[:], ap)
            return t

        w0t_sb = cload(w0t, (FEAT, HID), "w0t")
        w2t_sb = cload(w2t, (FEAT, HID), "w2t")
        w4t_sb = cload(w4t, (HID, OUT), "w4t")
        w6t_sb = cload(w6t, (HID, OUT), "w6t")
        b01_sb = cload(b01, (1, HID), "b01")
        b46_sb = cload(b46, (1, OUT), "b46")
        mask_sb = cload(bdmask, (128, 40), "mask")
        ident_sb = cload(ident, (128, 128), "ident")
        selft_sb = cload(self_t, (FEAT, bc), "selft")
        oht_sb = cload(one_hop_t, (FEAT, g), "oht")
        ohn_sb = sbig.tile([128, g], F32, tag="ohn")   # natural one_hop, chunked
        for t in range(nt1):
            nc.sync.dma_start(
                ohn_sb[:, 128 * t:128 * (t + 1)],
                one_hop[128 * t:128 * (t + 1), :],
            )

        ones_row = const.tile([1, ST_COLS], F32, tag="ones")
        zeros_row = const.tile([1, ST_COLS], F32, tag="zeros")
        nc.vector.memset(ones_row[:], 1.0)
        nc.vector.memset(zeros_row[:], 0.0)

        # e_flat = exp(l2f): the unnormalized soft2 weight for global row
        # 128*t + p at [p, t].
        l2f_sb = const.tile([128, r2 // 128], F32, tag="l2f")
        nc.sync.dma_start(l2f_sb[:], l2f)
        eflat_sb = const.tile([128, r2 // 128], F32, tag="eflat")
        nc.scalar.activation(eflat_sb[:], l2f_sb[:], AF.Exp)

        # ---- soft1 (normalized) + flat-transposed copy --------------------
        # (body below may be repeated for the timing harness)
        for _rep in range(repeat):
          d_s1 = dpool.tile([bc, HIST], F32, tag="ds1")
          d_rz2 = dpool.tile([bc, HIST], F32, tag="drz2")
          for j in range(nbt):
              l1t = spool.tile([128, HIST], F32, tag="l1")
              nc.sync.dma_start(l1t[:], l1[128 * j:128 * (j + 1), :])
              e1 = spool.tile([128, HIST], F32, tag="e1")
              nc.scalar.activation(e1[:], l1t[:], AF.Exp)
              z1 = spool.tile([128, 1], F32, tag="z1")
              nc.vector.reduce_sum(z1[:], e1[:], axis=mybir.AxisListType.X)
              rz1 = spool.tile([128, 1], F32, tag="rz1")
              nc.vector.reciprocal(rz1[:], z1[:])
              s1 = spool.tile([128, HIST], F32, tag="s1")
              nc.vector.tensor_scalar_mul(s1[:], e1[:], rz1[:])
              nc.sync.dma_start(d_s1[128 * j:128 * (j + 1), :], s1[:])

              # 1/Z for soft2, group-ordered [bc, 20]
              l2t = spool.tile([128, HIST * HIST], F32, tag="l2")
              nc.sync.dma_start(l2t[:], l2n[128 * j:128 * (j + 1), :])
              e2 = spool.tile([128, HIST * HIST], F32, tag="e2")
              nc.scalar.activation(e2[:], l2t[:], AF.Exp)
              z2 = spool.tile([128, HIST], F32, tag="z2")
              nc.vector.reduce_sum(
                  z2[:],
                  e2[:].rearrange("p (h k) -> p h k", k=HIST),
                  axis=mybir.AxisListType.X,
              )
              rz2 = spool.tile([128, HIST], F32, tag="rz2")
              nc.vector.reciprocal(rz2[:], z2[:])
              nc.sync.dma_start(d_rz2[128 * j:128 * (j + 1), :], rz2[:])

          # soft1 flat-transposed: [128, nt1], col t row p = soft1_flat[128t+p]
          s1v = spool.tile([nt1, 128], F32, tag="s1v")
          nc.sync.dma_start(
              s1v[:],
              d_s1[:].rearrange("a b -> (a b)").rearrange("(x y) -> x y", y=128),
          )
          pt = p_misc.tile([128, nt1], F32, tag="misc")
          nc.tensor.transpose(pt[:], s1v[:], ident_sb[:nt1, :nt1])
          s1flat_sb = const.tile([128, nt1], F32, tag="s1flat")
          nc.scalar.copy(s1flat_sb[:], pt[:])

          # 1/Z2 as a single row [1, g]
          rz2row = const.tile([1, g], F32, tag="rz2row")
          nc.sync.dma_start(rz2row[:1, :], d_rz2[:].rearrange("a b -> (a b)"))

          # Replicate 1/Z2 across partitions into SBUF (ones-column matmul).
          rz2rep_sb = sbig.tile([128, g], F32, tag="rz2rep")
          for s in range((g + ST_COLS - 1) // ST_COLS):
              cols = min(ST_COLS, g - ST_COLS * s)
              rp = p_misc.tile([128, cols], F32, tag="misc")
              nc.tensor.matmul(
                  rp[:], ones_row[:1, :128],
                  rz2row[:1, ST_COLS * s:ST_COLS * s + cols],
                  start=True, stop=True, skip_group_check=True,
              )
              nc.vector.tensor_copy(rz2rep_sb[:, ST_COLS * s:ST_COLS * s + cols], rp[:])

          # ---- phase 1: agg2T[f, group] ------------------------------------
          # BD tiles are built 5 passes at a time with one tensor_tensor:
          # bd5[p, j, m] = mask[p, j, m] * e_flat[p, t0+j]  (broadcast over m).
          agg2t_sb = sbig.tile([128, g], F32, tag="agg2t")
          for s in range(nst):
              cols = min(ST_COLS, g - ST_COLS * s)
              tps = cols * HIST // 128
              assert tps % 5 == 0
              pag = p_agg.tile([128, cols], F32, tag="agg")
              nc.tensor.matmul(
                  pag[:], ones_row[:1, :128], zeros_row[:1, :cols],
                  start=True, stop=False, skip_group_check=True,
              )
              for tl5 in range(0, tps, 5):
                  tg0 = (ST_COLS * HIST // 128) * s + tl5
                  bd5 = bdpool.tile([128, 40], F32, tag="bd5")
                  nc.vector.tensor_mul(
                      bd5[:].rearrange("p (j m) -> p j m", m=8),
                      mask_sb[:].rearrange("p (j m) -> p j m", m=8),
                      eflat_sb[:, tg0:tg0 + 5].to_broadcast([128, 5, 8]),
                  )
                  for j in range(5):
                      tl = tl5 + j
                      tg = tg0 + j
                      xt = xpool.tile([128, FEAT], F32, tag="x")
                      if mode != "nodma":
                          nc.sync.dma_start(xt[:], two_hop[128 * tg:128 * (tg + 1), :])
                      w = _phase_width((128 * tl) % 20)
                      gf = (128 * tl) // 20
                      if mode != "dmaonly":
                          nc.tensor.matmul(
                              pag[:, gf:gf + w], xt[:], bd5[:, 8 * j:8 * j + w],
                              start=False, stop=(tl == tps - 1), skip_group_check=True,
                          )
              nc.vector.tensor_mul(
                  agg2t_sb[:, ST_COLS * s:ST_COLS * s + cols], pag[:],
                  rz2rep_sb[:, ST_COLS * s:ST_COLS * s + cols],
              )

          # ---- phase 2: x_one_s (natural [g-part, hid]) --------------------
          xos_sb = sbig.tile([128, g], F32, tag="xos")
          for c in range(nt1):
              p2 = p_misc.tile([128, HID], F32, tag="misc")
              nc.tensor.matmul(
                  p2[:], ones_row[:1, :128], b01_sb[:1, :],
                  start=True, stop=False, skip_group_check=True,
              )
              nc.tensor.matmul(
                  p2[:], oht_sb[:, 128 * c:128 * (c + 1)], w0t_sb[:],
                  start=False, stop=False, skip_group_check=True,
              )
              nc.tensor.matmul(
                  p2[:], agg2t_sb[:, 128 * c:128 * (c + 1)], w2t_sb[:],
                  start=False, stop=True, skip_group_check=True,
              )
              nc.scalar.activation(xos_sb[:, 128 * c:128 * (c + 1)], p2[:], AF.Relu)

          # ---- layer-2 aggregations (soft1-weighted segment sums) ----------
          py = p_acc.tile([128, bc], F32, tag="py")
          pa1 = p_acc.tile([128, bc], F32, tag="pa1")
          nc.tensor.matmul(py[:], ones_row[:1, :128], zeros_row[:1, :bc],
                           start=True, stop=False, skip_group_check=True)
          nc.tensor.matmul(pa1[:], ones_row[:1, :128], zeros_row[:1, :bc],
                           start=True, stop=False, skip_group_check=True)
          assert nt1 % 5 == 0
          for t5 in range(0, nt1, 5):
              bd15 = bdpool.tile([128, 40], F32, tag="bd5")
              nc.vector.tensor_mul(
                  bd15[:].rearrange("p (j m) -> p j m", m=8),
                  mask_sb[:].rearrange("p (j m) -> p j m", m=8),
                  s1flat_sb[:, t5:t5 + 5].to_broadcast([128, 5, 8]),
              )
              for j in range(5):
                  t = t5 + j
                  w = _phase_width((128 * t) % 20)
                  bf = (128 * t) // 20
                  nc.tensor.matmul(
                      py[:, bf:bf + w], xos_sb[:, 128 * t:128 * (t + 1)],
                      bd15[:, 8 * j:8 * j + w],
                      start=False, stop=(t == nt1 - 1), skip_group_check=True,
                  )
                  nc.tensor.matmul(
                      pa1[:, bf:bf + w], ohn_sb[:, 128 * t:128 * (t + 1)],
                      bd15[:, 8 * j:8 * j + w],
                      start=False, stop=(t == nt1 - 1), skip_group_check=True,
                  )
          yt_sb = sbig.tile([128, bc], F32, tag="yt")
          nc.scalar.copy(yt_sb[:], py[:])
          a1t_sb = sbig.tile([128, bc], F32, tag="a1t")
          nc.scalar.copy(a1t_sb[:], pa1[:])

          # ---- x_s_one (transposed [hid, b]) -------------------------------
          pxs = p_acc.tile([128, bc], F32, tag="pxs")
          nc.tensor.matmul(pxs[:], b01_sb[:1, :], ones_row[:1, :bc],
                           start=True, stop=False, skip_group_check=True)
          nc.tensor.matmul(pxs[:], w0t_sb[:], selft_sb[:],
                           start=False, stop=False, skip_group_check=True)
          nc.tensor.matmul(pxs[:], w2t_sb[:], a1t_sb[:],
                           start=False, stop=True, skip_group_check=True)
          xst_sb = sbig.tile([128, bc], F32, tag="xst")
          nc.scalar.activation(xst_sb[:], pxs[:], AF.Relu)

          # ---- final layer --------------------------------------------------
          for j in range(nbt):
              po = p_misc.tile([128, OUT], F32, tag="misc")
              nc.tensor.matmul(po[:], ones_row[:1, :128], b46_sb[:1, :],
                               start=True, stop=False, skip_group_check=True)
              nc.tensor.matmul(po[:], xst_sb[:, 128 * j:128 * (j + 1)], w4t_sb[:],
                               start=False, stop=False, skip_group_check=True)
              nc.tensor.matmul(po[:], yt_sb[:, 128 * j:128 * (j + 1)], w6t_sb[:],
                               start=False, stop=True, skip_group_check=True)
              ot = spool.tile([128, OUT], F32, tag="ot")
              nc.scalar.copy(ot[:], po[:])
              nc.sync.dma_start(out_d[128 * j:128 * (j + 1), :], ot[:])

    nc.compile()
    return nc


def make_in_maps(inputs: dict, bc: int = BC, ncores: int = NCORES):
    """Host-side shard + auxiliary layout prep. Returns list of per-core dicts."""
    f32 = np.float32
    self_feat = np.asarray(inputs["self_feat"], f32)
    one_hop = np.asarray(inputs["one_hop_feat"], f32)
    two_hop = np.asarray(inputs["two_hop_feat"], f32)
    e_time = np.asarray(inputs["e_time"], f32)
    his_time = np.asarray(inputs["his_time"], f32)
    his_his = np.asarray(inputs["his_his_time"], f32)
    W0 = np.asarray(inputs["W0"], f32)
    b0 = np.asarray(inputs["b0"], f32)
    W2 = np.asarray(inputs["W2"], f32)
    b2 = np.asarray(inputs["b2"], f32)
    W4 = np.asarray(inputs["W4"], f32)
    b4 = np.asarray(inputs["b4"], f32)
    W6 = np.asarray(inputs["W6"], f32)
    b6 = np.asarray(inputs["b6"], f32)
    delta = float(np.asarray(inputs["delta"]).reshape(-1)[0])

    g = bc * HIST
    r2 = g * HIST
    C = np.ascontiguousarray
    shared = {
        "w0t": C(W0.T), "w2t": C(W2.T), "w4t": C(W4.T), "w6t": C(W6.T),
        "b01": (b0 + b2).reshape(1, HID).copy(),
        "b46": (b4 + b6).reshape(1, OUT).copy(),
        "bdmask": build_bdmask(),
        "ident": np.eye(128, dtype=f32),
    }
    maps = []
    for c in range(ncores):
        bs = slice(c * bc, (c + 1) * bc)
        oh = one_hop[c * g:(c + 1) * g]
        l1 = delta * (his_time[bs] - e_time[bs, None])
        l2 = delta * (his_his[bs] - his_time[bs, :, None])   # [bc, H, H]
        maps.append({
            "two_hop": C(two_hop[c * r2:(c + 1) * r2]),
            "one_hop": C(oh),
            "one_hop_t": C(oh.T),
            "self_t": C(self_feat[bs].T),
            "l1": C(l1),
            "l2n": C(l2.reshape(bc, HIST * HIST)),
            "l2f": C(l2.reshape(r2 // 128, 128).T),
            **shared,
        })
    return maps


def kernel(**inputs) -> np.ndarray:
    from concourse.bass_utils import run_bass_kernel_spmd

    nc = build_program(BC)
    in_maps = make_in_maps(inputs)
    res = run_bass_kernel_spmd(nc, in_maps, core_ids=list(range(NCORES)))
    return np.concatenate([res.results[c]["out"] for c in range(NCORES)], axis=0)



# revision 3
# speedup vs baseline: 5.0150x; 5.0150x over previous
"""Trainium2 Bass kernel for the DGNN message-passing module (fp16 rev).

Contract: kernel(**inputs) takes the FULL unsharded inputs and returns
the full [2048, 64] float32 output.  Internally the leading B (event)
dimension is sharded across 8 NeuronCores (pure data parallel); small
weights are replicated.

Math (per core, b=256, H=20, FEAT=HID=128, OUT=64):
  soft1 = softmax(-delta*(e_time[:,None]-his_time), axis=1)
  soft2 = softmax(-delta*(his_time[:,:,None]-his_his_time), axis=2)
  agg1[b]   = sum_h soft1[b,h] * one_hop[b,h,:]          (linearity pull-out)
  agg2[b,h] = sum_k soft2[b,h,k] * two_hop[b,h,k,:]
  x_s_one = relu(self@W0.T + agg1@W2.T + b0+b2)
  x_one_s = relu(one_hop@W0.T + agg2@W2.T + b0+b2)
  y[b]    = sum_h soft1[b,h] * x_one_s[b,h,:]
  out     = x_s_one@W4.T + y@W6.T + b4+b6

The dominant cost is streaming two_hop; it is shipped as fp16 in a
[128, 800*128] row-tiled layout (row 128t+p at [p, 128t+f]) so every
DMA is a [128, wide-contiguous] block.  The weighted segment-sum agg2
runs on the tensor engine: per 128-row tile (fp16 stationary, FWL) a
[128, <=8] "block diagonal" 0/1-mask*exp(logit) moving tile accumulates
group columns in PSUM; softmax 1/Z is folded into the PSUM eviction.
fp16 matmuls run 1 cycle/row (fp32 = 4 + LOW/HIGH instruction pairs),
so this rev halves DMA bytes and ~4x's the tensor-engine throughput of
the fp32 baseline.
"""

import sys

import numpy as np

sys.path.insert(0, "/opt/trn_rl_repo")

B, HIST, FEAT, HID, OUT = 2048, 20, 128, 128, 64
NCORES = 8
BC = B // NCORES          # 256 events per core
G = BC * HIST             # 5120 (b,h) groups per core
R2 = G * HIST             # 102400 two-hop rows per core
NT2 = R2 // 128           # 800 two_hop row-tiles
NT1 = G // 128            # 40 one_hop row-tiles
ST_COLS = 512             # PSUM group-columns per supertile (1 bank of fp32)
TPS = ST_COLS * HIST // 128   # 80 two_hop tiles per supertile
CH = 10                   # two_hop tiles per DMA chunk (2 bd-builds each)

# (128*t) % 20 cycles with period 5; per-phase mask width (# groups touched
# by a 128-row pass).
PHIS = [0, 8, 16, 4, 12]


def _phase_width(phi: int) -> int:
    return (phi + 127) // 20 + 1


def build_bdmask() -> np.ndarray:
    """[128, 40] = 5 masks of [128, 8]: mask[p, 8*i + m] = 1 if (phi_i+p)//20 == m."""
    m = np.zeros((128, 40), np.float16)
    for i, phi in enumerate(PHIS):
        for p in range(128):
            m[p, 8 * i + (phi + p) // 20] = 1.0
    return m


def build_program(bc: int = BC, repeat: int = 1, mode: str = "full"):
    """Build the SPMD Bass program (one NeuronCore's view). Returns nc.

    repeat>1 duplicates the whole compute body (timing harness only).
    mode: "full" | "dmaonly" (stream two_hop, skip phase-1 matmuls) |
    "nodma" (skip the two_hop stream DMAs)."""
    import concourse.bass as bass
    import concourse.tile as tile
    from concourse import bacc, mybir
    from contextlib import ExitStack

    F32 = mybir.dt.float32
    F16 = mybir.dt.float16
    AF = mybir.ActivationFunctionType
    g = bc * HIST
    r2 = g * HIST
    nbt = bc // 128              # b-chunks (2)
    nt1 = g // 128               # 128-row passes over one_hop / x_one_s (40)
    nt2 = r2 // 128
    nst = (g + ST_COLS - 1) // ST_COLS
    nch = nt2 // CH              # two_hop DMA chunks (80)

    nc = bacc.Bacc("TRN2", target_bir_lowering=False, debug=False)

    def din(name, shape, dt=F16):
        return nc.dram_tensor(name, list(shape), dt, kind="ExternalInput").ap()

    two_hop = din("two_hop", (128, nt2 * FEAT))       # tiled [p, 128t+f]
    one_hop_n = din("one_hop_n", (128, nt1 * FEAT))   # tiled natural
    one_hop_t = din("one_hop_t", (FEAT, g))           # transposed
    self_t = din("self_t", (FEAT, bc))
    l1 = din("l1", (bc, HIST), F32)       # delta*(his_time - e_time[:,None])
    l2n = din("l2n", (bc, HIST * HIST))   # delta*(his_his - his_time[:,:,None])
    l2f = din("l2f", (128, nt2))          # same, flat-transposed [p, t] = v[128t+p]
    w0t = din("w0t", (FEAT, HID))
    w2t = din("w2t", (FEAT, HID))
    w4t = din("w4t", (HID, OUT))
    w6t = din("w6t", (HID, OUT))
    b01 = din("b01", (1, HID))
    b46 = din("b46", (1, OUT))
    bdmask = din("bdmask", (128, 40))
    ident = din("ident", (128, 128), F32)
    out_d = nc.dram_tensor("out", [bc, OUT], F32, kind="ExternalOutput").ap()

    with tile.TileContext(nc) as tc, ExitStack() as ctx:
        const = ctx.enter_context(tc.tile_pool(name="const", bufs=1))
        sbig = ctx.enter_context(tc.tile_pool(name="sbig", bufs=1))
        xpool = ctx.enter_context(tc.tile_pool(name="xp", bufs=6))
        bdpool = ctx.enter_context(tc.tile_pool(name="bdp", bufs=4))
        spool = ctx.enter_context(tc.tile_pool(name="sp", bufs=4))
        dpool = ctx.enter_context(tc.tile_pool(name="dram", bufs=1, space="DRAM"))
        p_agg = ctx.enter_context(tc.tile_pool(name="pagg", bufs=2, space="PSUM"))
        p_misc = ctx.enter_context(tc.tile_pool(name="pmisc", bufs=2, space="PSUM"))
        p_acc = ctx.enter_context(tc.tile_pool(name="pacc", bufs=1, space="PSUM"))

        def cload(ap, shape, tag, dt=F16):
            t = const.tile(list(shape), dt, tag=tag)
            nc.sync.dma_start(t[:], ap)
            return t

        w0t_sb = cload(w0t, (FEAT, HID), "w0t")
        w2t_sb = cload(w2t, (FEAT, HID), "w2t")
        w4t_sb = cload(w4t, (HID, OUT), "w4t")
        w6t_sb = cload(w6t, (HID, OUT), "w6t")
        b01_sb = cload(b01, (1, HID), "b01")
        b46_sb = cload(b46, (1, OUT), "b46")
        mask_sb = cload(bdmask, (128, 40), "mask")
        ident_sb = cload(ident, (128, 128), "ident", F32)
        selft_sb = cload(self_t, (FEAT, bc), "selft")
        oht_sb = cload(one_hop_t, (FEAT, g), "oht")
        ohn_sb = cload(one_hop_n, (128, nt1 * FEAT), "ohn")

        ones_row = const.tile([1, ST_COLS], F16, tag="ones")
        zeros_row = const.tile([1, ST_COLS], F16, tag="zeros")
        nc.vector.memset(ones_row[:], 1.0)
        nc.vector.memset(zeros_row[:], 0.0)

        # e_flat = exp(l2f): the unnormalized soft2 weight for global row
        # 128*t + p at [p, t].
        l2f_sb = const.tile([128, nt2], F16, tag="l2f")
        nc.sync.dma_start(l2f_sb[:], l2f)
        eflat_sb = const.tile([128, nt2], F16, tag="eflat")
        nc.scalar.activation(eflat_sb[:], l2f_sb[:], AF.Exp)

        # ---- soft1 (normalized) + flat-transposed copy --------------------
        # (body below may be repeated for the timing harness)
        for _rep in range(repeat):
          d_s1 = dpool.tile([bc, HIST], F32, tag="ds1")
          d_rz2 = dpool.tile([bc, HIST], F16, tag="drz2")
          for j in range(nbt):
              l1t = spool.tile([128, HIST], F32, tag="l1")
              nc.sync.dma_start(l1t[:], l1[128 * j:128 * (j + 1), :])
              e1 = spool.tile([128, HIST], F32, tag="e1")
              nc.scalar.activation(e1[:], l1t[:], AF.Exp)
              z1 = spool.tile([128, 1], F32, tag="z1")
              nc.vector.reduce_sum(z1[:], e1[:], axis=mybir.AxisListType.X)
              rz1 = spool.tile([128, 1], F32, tag="rz1")
              nc.vector.reciprocal(rz1[:], z1[:])
              s1 = spool.tile([128, HIST], F32, tag="s1")
              nc.vector.tensor_scalar_mul(s1[:], e1[:], rz1[:])
              nc.sync.dma_start(d_s1[128 * j:128 * (j + 1), :], s1[:])

              # 1/Z for soft2, group-ordered [bc, 20]
              l2t = spool.tile([128, HIST * HIST], F16, tag="l2")
              nc.sync.dma_start(l2t[:], l2n[128 * j:128 * (j + 1), :])
              e2 = spool.tile([128, HIST * HIST], F32, tag="e2")
              nc.scalar.activation(e2[:], l2t[:], AF.Exp)
              z2 = spool.tile([128, HIST], F32, tag="z2")
              nc.vector.reduce_sum(
                  z2[:],
                  e2[:].rearrange("p (h k) -> p h k", k=HIST),
                  axis=mybir.AxisListType.X,
              )
              rz2 = spool.tile([128, HIST], F32, tag="rz2")
              nc.vector.reciprocal(rz2[:], z2[:])
              rz2h = spool.tile([128, HIST], F16, tag="rz2h")
              nc.scalar.copy(rz2h[:], rz2[:])
              nc.sync.dma_start(d_rz2[128 * j:128 * (j + 1), :], rz2h[:])

          # soft1 flat-transposed: [128, nt1], col t row p = soft1_flat[128t+p]
          s1v = spool.tile([nt1, 128], F32, tag="s1v")
          nc.sync.dma_start(
              s1v[:],
              d_s1[:].rearrange("a b -> (a b)").rearrange("(x y) -> x y", y=128),
          )
          pt = p_misc.tile([128, nt1], F32, tag="misc")
          nc.tensor.transpose(pt[:], s1v[:], ident_sb[:nt1, :nt1])
          s1flat_sb = const.tile([128, nt1], F16, tag="s1flat")
          nc.scalar.copy(s1flat_sb[:], pt[:])

          # 1/Z2 as a single row [1, g]
          rz2row = const.tile([1, g], F16, tag="rz2row")
          nc.sync.dma_start(rz2row[:1, :], d_rz2[:].rearrange("a b -> (a b)"))

          # Replicate 1/Z2 across partitions into SBUF (ones-column matmul).
          rz2rep_sb = sbig.tile([128, g], F32, tag="rz2rep")
          for s in range((g + ST_COLS - 1) // ST_COLS):
              cols = min(ST_COLS, g - ST_COLS * s)
              rp = p_misc.tile([128, cols], F32, tag="misc")
              nc.tensor.matmul(
                  rp[:], ones_row[:1, :128],
                  rz2row[:1, ST_COLS * s:ST_COLS * s + cols],
                  start=True, stop=True, skip_group_check=True,
              )
              nc.vector.tensor_copy(rz2rep_sb[:, ST_COLS * s:ST_COLS * s + cols], rp[:])

          # ---- phase 1: agg2T[f, group] ------------------------------------
          # Stream two_hop in CH-tile chunks; per 5 tiles one tensor_tensor
          # builds the bd tiles: bd5[p, j, m] = mask[p, j, m] * e_flat[p, t0+j].
          agg2t_sb = sbig.tile([128, g], F16, tag="agg2t")
          pag = None
          for c in range(nch):
              xt = xpool.tile([128, CH * FEAT], F16, tag="x")
              if mode != "nodma":
                  eng = nc.sync if (c % 2 == 0) else nc.scalar
                  eng.dma_start(xt[:], two_hop[:, CH * FEAT * c:CH * FEAT * (c + 1)])
              for h5 in range(CH // 5):
                  tg0 = CH * c + 5 * h5          # global tile idx of this 5-group
                  tl0 = tg0 % TPS                # tile idx within supertile
                  s = tg0 // TPS                 # supertile idx
                  if tl0 == 0:
                      pag = p_agg.tile([128, ST_COLS], F32, tag="agg")
                      nc.tensor.matmul(
                          pag[:], ones_row[:1, :128], zeros_row[:1, :ST_COLS],
                          start=True, stop=False, skip_group_check=True,
                      )
                  bd5 = bdpool.tile([128, 40], F16, tag="bd5")
                  nc.vector.tensor_mul(
                      bd5[:].rearrange("p (j m) -> p j m", m=8),
                      mask_sb[:].rearrange("p (j m) -> p j m", m=8),
                      eflat_sb[:, tg0:tg0 + 5].to_broadcast([128, 5, 8]),
                  )
                  if mode == "dmaonly":
                      continue
                  for j in range(5):
                      tl = tl0 + j
                      w = _phase_width((128 * tl) % 20)
                      gf = (128 * tl) // 20
                      nc.tensor.matmul(
                          pag[:, gf:gf + w],
                          xt[:, FEAT * (5 * h5 + j):FEAT * (5 * h5 + j + 1)],
                          bd5[:, 8 * j:8 * j + w],
                          start=False, stop=(tl == TPS - 1), skip_group_check=True,
                      )
                  if tl0 + 5 == TPS:
                      nc.vector.tensor_mul(
                          agg2t_sb[:, ST_COLS * s:ST_COLS * (s + 1)], pag[:],
                          rz2rep_sb[:, ST_COLS * s:ST_COLS * (s + 1)],
                      )
          if mode == "dmaonly":
              continue

          # ---- phase 2: x_one_s (natural [g-part, hid]) --------------------
          xos_sb = sbig.tile([128, g], F16, tag="xos")
          for c in range(nt1):
              p2 = p_misc.tile([128, HID], F32, tag="misc")
              nc.tensor.matmul(
                  p2[:], ones_row[:1, :128], b01_sb[:1, :],
                  start=True, stop=False, skip_group_check=True,
              )
              nc.tensor.matmul(
                  p2[:], oht_sb[:, 128 * c:128 * (c + 1)], w0t_sb[:],
                  start=False, stop=False, skip_group_check=True,
              )
              nc.tensor.matmul(
                  p2[:], agg2t_sb[:, 128 * c:128 * (c + 1)], w2t_sb[:],
                  start=False, stop=True, skip_group_check=True,
              )
              nc.scalar.activation(xos_sb[:, 128 * c:128 * (c + 1)], p2[:], AF.Relu)

          # ---- layer-2 aggregations (soft1-weighted segment sums) ----------
          py = p_acc.tile([128, bc], F32, tag="py")
          pa1 = p_acc.tile([128, bc], F32, tag="pa1")
          nc.tensor.matmul(py[:], ones_row[:1, :128], zeros_row[:1, :bc],
                           start=True, stop=False, skip_group_check=True)
          nc.tensor.matmul(pa1[:], ones_row[:1, :128], zeros_row[:1, :bc],
                           start=True, stop=False, skip_group_check=True)
          assert nt1 % 5 == 0
          for t5 in range(0, nt1, 5):
              bd15 = bdpool.tile([128, 40], F16, tag="bd5")
              nc.vector.tensor_mul(
                  bd15[:].rearrange("p (j m) -> p j m", m=8),
                  mask_sb[:].rearrange("p (j m) -> p j m", m=8),
                  s1flat_sb[:, t5:t5 + 5].to_broadcast([128, 5, 8]),
              )
              for j in range(5):
                  t = t5 + j
                  w = _phase_width((128 * t) % 20)
                  bf = (128 * t) // 20
                  nc.tensor.matmul(
                      py[:, bf:bf + w], xos_sb[:, 128 * t:128 * (t + 1)],
                      bd15[:, 8 * j:8 * j + w],
                      start=False, stop=(t == nt1 - 1), skip_group_check=True,
                  )
                  nc.tensor.matmul(
                      pa1[:, bf:bf + w], ohn_sb[:, 128 * t:128 * (t + 1)],
                      bd15[:, 8 * j:8 * j + w],
                      start=False, stop=(t == nt1 - 1), skip_group_check=True,
                  )
          yt_sb = sbig.tile([128, bc], F16, tag="yt")
          nc.scalar.copy(yt_sb[:], py[:])
          a1t_sb = sbig.tile([128, bc], F16, tag="a1t")
          nc.scalar.copy(a1t_sb[:], pa1[:])

          # ---- x_s_one (transposed [hid, b]) -------------------------------
          pxs = p_acc.tile([128, bc], F32, tag="pxs")
          nc.tensor.matmul(pxs[:], b01_sb[:1, :], ones_row[:1, :bc],
                           start=True, stop=False, skip_group_check=True)
          nc.tensor.matmul(pxs[:], w0t_sb[:], selft_sb[:],
                           start=False, stop=False, skip_group_check=True)
          nc.tensor.matmul(pxs[:], w2t_sb[:], a1t_sb[:],
                           start=False, stop=True, skip_group_check=True)
          xst_sb = sbig.tile([128, bc], F16, tag="xst")
          nc.scalar.activation(xst_sb[:], pxs[:], AF.Relu)

          # ---- final layer --------------------------------------------------
          for j in range(nbt):
              po = p_misc.tile([128, OUT], F32, tag="misc")
              nc.tensor.matmul(po[:], ones_row[:1, :128], b46_sb[:1, :],
                               start=True, stop=False, skip_group_check=True)
              nc.tensor.matmul(po[:], xst_sb[:, 128 * j:128 * (j + 1)], w4t_sb[:],
                               start=False, stop=False, skip_group_check=True)
              nc.tensor.matmul(po[:], yt_sb[:, 128 * j:128 * (j + 1)], w6t_sb[:],
                               start=False, stop=True, skip_group_check=True)
              ot = spool.tile([128, OUT], F32, tag="ot")
              nc.scalar.copy(ot[:], po[:])
              nc.sync.dma_start(out_d[128 * j:128 * (j + 1), :], ot[:])

    nc.compile()
    return nc


def make_in_maps(inputs: dict, bc: int = BC, ncores: int = NCORES):
    """Host-side shard + auxiliary layout prep. Returns list of per-core dicts."""
    f32 = np.float32
    f16 = np.float16
    self_feat = np.asarray(inputs["self_feat"], f32)
    one_hop = np.asarray(inputs["one_hop_feat"], f32)
    two_hop = np.asarray(inputs["two_hop_feat"], f32)
    e_time = np.asarray(inputs["e_time"], f32)
    his_time = np.asarray(inputs["his_time"], f32)
    his_his = np.asarray(inputs["his_his_time"], f32)
    W0 = np.asarray(inputs["W0"], f32)
    b0 = np.asarray(inputs["b0"], f32)
    W2 = np.asarray(inputs["W2"], f32)
    b2 = np.asarray(inputs["b2"], f32)
    W4 = np.asarray(inputs["W4"], f32)
    b4 = np.asarray(inputs["b4"], f32)
    W6 = np.asarray(inputs["W6"], f32)
    b6 = np.asarray(inputs["b6"], f32)
    delta = float(np.asarray(inputs["delta"]).reshape(-1)[0])

    g = bc * HIST
    r2 = g * HIST

    def tiled(x):
        """[N*128, 128] -> [128, N*128] with row 128t+p at [p, 128t+f]."""
        n = x.shape[0] // 128
        return np.ascontiguousarray(
            x.reshape(n, 128, FEAT).transpose(1, 0, 2).reshape(128, n * FEAT)
        )

    C = np.ascontiguousarray
    shared = {
        "w0t": C(W0.T).astype(f16), "w2t": C(W2.T).astype(f16),
        "w4t": C(W4.T).astype(f16), "w6t": C(W6.T).astype(f16),
        "b01": (b0 + b2).reshape(1, HID).astype(f16),
        "b46": (b4 + b6).reshape(1, OUT).astype(f16),
        "bdmask": build_bdmask(),
        "ident": np.eye(128, dtype=f32),
    }
    maps = []
    for c in range(ncores):
        bs = slice(c * bc, (c + 1) * bc)
        oh = one_hop[c * g:(c + 1) * g]
        l1 = delta * (his_time[bs] - e_time[bs, None])
        l2 = delta * (his_his[bs] - his_time[bs, :, None])   # [bc, H, H]
        maps.append({
            "two_hop": tiled(two_hop[c * r2:(c + 1) * r2].astype(f16)),
            "one_hop_n": tiled(oh.astype(f16)),
            "one_hop_t": C(oh.T).astype(f16),
            "self_t": C(self_feat[bs].T).astype(f16),
            "l1": C(l1),
            "l2n": C(l2.reshape(bc, HIST * HIST)).astype(f16),
            "l2f": C(l2.reshape(r2 // 128, 128).T).astype(f16),
            **shared,
        })
    return maps


def kernel(**inputs) -> np.ndarray:
    from concourse.bass_utils import run_bass_kernel_spmd

    nc = build_program(BC)
    in_maps = make_in_maps(inputs)
    res = run_bass_kernel_spmd(nc, in_maps, core_ids=list(range(NCORES)))
    return np.concatenate([res.results[c]["out"] for c in range(NCORES)], axis=0)


# revision 15
# speedup vs baseline: 5.3291x; 1.0626x over previous
"""Trainium2 Bass kernel for the DGNN message-passing module (fp16 rev2).

Contract: kernel(**inputs) takes the FULL unsharded inputs and returns
the full [2048, 64] float32 output.  Internally the leading B (event)
dimension is sharded across 8 NeuronCores (pure data parallel); small
weights are replicated.

Math (per core, b=256, H=20, FEAT=HID=128, OUT=64):
  soft1 = softmax(-delta*(e_time[:,None]-his_time), axis=1)
  soft2 = softmax(-delta*(his_time[:,:,None]-his_his_time), axis=2)
  agg1[b]   = sum_h soft1[b,h] * one_hop[b,h,:]          (linearity pull-out)
  agg2[b,h] = sum_k soft2[b,h,k] * two_hop[b,h,k,:]
  x_s_one = relu(self@W0.T + agg1@W2.T + b0+b2)
  x_one_s = relu(one_hop@W0.T + agg2@W2.T + b0+b2)
  y[b]    = sum_h soft1[b,h] * x_one_s[b,h,:]
  out     = x_s_one@W4.T + y@W6.T + b4+b6

Strategy: stream two_hop as fp16 in a [128, 800*128] row-tiled layout
(row 128t+p at [p, 128t+f]) in 20-tile chunks on a dedicated DMA issue
queue (Sync engine); everything else DMAs on the Activation engine with
the bd-weight inputs (l2f) kicked first so the tensor engine starts
within ~2us of the first chunk landing.  agg2 is a tensor-engine
weighted segment-sum: per 128-row tile (fp16 stationary, FWL) a
[128, <=8] "block diagonal" mask*exp(logit) moving tile accumulates
group columns in PSUM; softmax 1/Z2 is folded into the PSUM eviction
(GpSimd multiply by a replicated 1/Z row).  soft1 is likewise applied
unnormalized via exp(l1) masked matmuls with 1/Z1 folded into the y/a1
evictions.  Phase-2 (x_one_s) and the layer-2 aggregations are
interleaved into the stream at supertile boundaries so the PE tail
after the last DMA is only the final ~2us.
"""

import sys

import numpy as np

sys.path.insert(0, "/opt/trn_rl_repo")

B, HIST, FEAT, HID, OUT = 2048, 20, 128, 128, 64
NCORES = 8
BC = B // NCORES          # 256 events per core
G = BC * HIST             # 5120 (b,h) groups per core
R2 = G * HIST             # 102400 two-hop rows per core
NT2 = R2 // 128           # 800 two_hop row-tiles
NT1 = G // 128            # 40 one_hop row-tiles
ST_COLS = 512             # PSUM group-columns per supertile (1 bank of fp32)
TPS = ST_COLS * HIST // 128   # 80 two_hop tiles per supertile
CH = 20                   # two_hop tiles per DMA chunk (4 bd-builds each)

# (128*t) % 20 cycles with period 5; per-phase mask width (# groups touched
# by a 128-row pass).
PHIS = [0, 8, 16, 4, 12]


def _phase_width(phi: int) -> int:
    return (phi + 127) // 20 + 1


def build_bdmask() -> np.ndarray:
    """[128, 40] = 5 masks of [128, 8]: mask[p, 8*i + m] = 1 if (phi_i+p)//20 == m."""
    m = np.zeros((128, 40), np.float16)
    for i, phi in enumerate(PHIS):
        for p in range(128):
            m[p, 8 * i + (phi + p) // 20] = 1.0
    return m


def build_program(bc: int = BC, repeat: int = 1, mode: str = "full"):
    """Build the SPMD Bass program (one NeuronCore's view). Returns nc.

    repeat>1 duplicates the whole compute body (timing harness only).
    mode: "full" | "dmaonly" (stream two_hop, skip phase-1 matmuls) |
    "nodma" (skip the two_hop stream DMAs)."""
    import concourse.bass as bass
    import concourse.tile as tile
    from concourse import bacc, mybir
    from contextlib import ExitStack

    F32 = mybir.dt.float32
    F16 = mybir.dt.float16
    AF = mybir.ActivationFunctionType
    g = bc * HIST
    r2 = g * HIST
    nbt = bc // 128              # b-chunks (2)
    nt1 = g // 128               # 128-row passes over one_hop / x_one_s (40)
    nt2 = r2 // 128
    nst = (g + ST_COLS - 1) // ST_COLS   # supertiles (10)
    nch = nt2 // CH              # two_hop DMA chunks (40)
    cps = TPS // CH              # chunks per supertile (4)
    xcs = ST_COLS // 128         # xos chunks per supertile (4)

    nc = bacc.Bacc("TRN2", target_bir_lowering=False, debug=False)

    def din(name, shape, dt=F16):
        return nc.dram_tensor(name, list(shape), dt, kind="ExternalInput").ap()

    two_hop = din("two_hop", (128, nt2 * FEAT))       # tiled [p, 128t+f]
    one_hop_n = din("one_hop_n", (128, nt1 * FEAT))   # tiled natural
    one_hop_t = din("one_hop_t", (FEAT, g))           # transposed
    self_t = din("self_t", (FEAT, bc))
    l1f = din("l1f", (128, nt1))          # soft1 logits flat-tiled [p,t]=v[128t+p]
    l1n = din("l1n", (bc, HIST), F32)     # soft1 logits natural (for Z1)
    l2n = din("l2n", (bc, HIST * HIST))   # delta*(his_his - his_time[:,:,None])
    l2f = din("l2f", (128, nt2))          # same, flat-transposed [p, t] = v[128t+p]
    w0t = din("w0t", (FEAT, HID))
    w2t = din("w2t", (FEAT, HID))
    w4t = din("w4t", (HID, OUT))
    w6t = din("w6t", (HID, OUT))
    b01 = din("b01", (1, HID))
    b46 = din("b46", (1, OUT))
    bdmask = din("bdmask", (128, 40))
    out_d = nc.dram_tensor("out", [bc, OUT], F32, kind="ExternalOutput").ap()

    with tile.TileContext(nc) as tc, ExitStack() as ctx:
        const = ctx.enter_context(tc.tile_pool(name="const", bufs=1))
        sbig = ctx.enter_context(tc.tile_pool(name="sbig", bufs=1))
        xpool = ctx.enter_context(tc.tile_pool(name="xp", bufs=4))
        bdpool = ctx.enter_context(tc.tile_pool(name="bdp", bufs=4))
        spool = ctx.enter_context(tc.tile_pool(name="sp", bufs=4))
        dpool = ctx.enter_context(tc.tile_pool(name="dram", bufs=1, space="DRAM"))
        p_agg = ctx.enter_context(tc.tile_pool(name="pagg", bufs=2, space="PSUM"))
        p_misc = ctx.enter_context(tc.tile_pool(name="pmisc", bufs=2, space="PSUM"))
        p_acc = ctx.enter_context(tc.tile_pool(name="pacc", bufs=1, space="PSUM"))

        # ---- priority DMAs (Activation engine queue): bd-weight inputs ----
        l2f_sb = const.tile([128, nt2], F16, tag="l2f")
        nc.scalar.dma_start(l2f_sb[:], l2f)
        mask_sb = const.tile([128, 40], F16, tag="mask")
        nc.scalar.dma_start(mask_sb[:], bdmask)
        l1f_sb = const.tile([128, nt1], F16, tag="l1f")
        nc.scalar.dma_start(l1f_sb[:], l1f)

        # e_flat = exp(l2f): unnormalized soft2 weight for row 128t+p at [p,t]
        eflat_sb = const.tile([128, nt2], F16, tag="eflat")
        nc.scalar.activation(eflat_sb[:], l2f_sb[:], AF.Exp)
        # e1f = exp(l1f): unnormalized soft1 weight, flat-tiled
        e1f_sb = const.tile([128, nt1], F16, tag="e1f")
        nc.scalar.activation(e1f_sb[:], l1f_sb[:], AF.Exp)

        l1ts, l2ts = [], []
        for j in range(nbt):
            l1t = spool.tile([128, HIST], F32, tag=f"l1_{j}")
            nc.scalar.dma_start(l1t[:], l1n[128 * j:128 * (j + 1), :])
            l1ts.append(l1t)
            l2t = spool.tile([128, HIST * HIST], F16, tag=f"l2_{j}")
            nc.scalar.dma_start(l2t[:], l2n[128 * j:128 * (j + 1), :])
            l2ts.append(l2t)

        ones_row = const.tile([1, ST_COLS], F16, tag="ones")
        zeros_row = const.tile([1, ST_COLS], F16, tag="zeros")
        nc.vector.memset(ones_row[:], 1.0)
        nc.vector.memset(zeros_row[:], 0.0)

        # (body below may be repeated for the timing harness)
        for _rep in range(repeat):
          # ---- softmax normalizers (flat rows via tiny DRAM round-trip) ---
          d_rz1 = dpool.tile([bc, 1], F16, tag="drz1")
          d_rz2 = dpool.tile([bc, HIST], F16, tag="drz2")
          for j in range(nbt):
              e1 = spool.tile([128, HIST], F32, tag="e1")
              nc.scalar.activation(e1[:], l1ts[j][:], AF.Exp)
              z1 = spool.tile([128, 1], F32, tag="z1")
              nc.vector.reduce_sum(z1[:], e1[:], axis=mybir.AxisListType.X)
              rz1 = spool.tile([128, 1], F16, tag="rz1")
              with nc.allow_low_precision(reason="1/Z in fp16: 5e-4 rel, gate is 2e-2"):
                  nc.vector.reciprocal(rz1[:], z1[:])
              nc.scalar.dma_start(d_rz1[128 * j:128 * (j + 1), :], rz1[:])

              e2 = spool.tile([128, HIST * HIST], F32, tag="e2")
              nc.scalar.activation(e2[:], l2ts[j][:], AF.Exp)
              z2 = spool.tile([128, HIST], F32, tag="z2")
              nc.vector.reduce_sum(
                  z2[:],
                  e2[:].rearrange("p (h k) -> p h k", k=HIST),
                  axis=mybir.AxisListType.X,
              )
              rz2 = spool.tile([128, HIST], F16, tag="rz2")
              with nc.allow_low_precision(reason="1/Z in fp16: 5e-4 rel, gate is 2e-2"):
                  nc.vector.reciprocal(rz2[:], z2[:])
              nc.scalar.dma_start(d_rz2[128 * j:128 * (j + 1), :], rz2[:])

          rz1row = const.tile([1, bc], F16, tag="rz1row")
          nc.scalar.dma_start(rz1row[:1, :], d_rz1[:].rearrange("a b -> (a b)"))
          rz2row = const.tile([1, g], F16, tag="rz2row")
          nc.scalar.dma_start(rz2row[:1, :], d_rz2[:].rearrange("a b -> (a b)"))

          # ---- remaining constants (Activation engine queue) --------------
          def cload(ap, shape, tag, dt=F16):
              t = const.tile(list(shape), dt, tag=tag)
              nc.scalar.dma_start(t[:], ap)
              return t

          w0t_sb = cload(w0t, (FEAT, HID), "w0t")
          w2t_sb = cload(w2t, (FEAT, HID), "w2t")
          w4t_sb = cload(w4t, (HID, OUT), "w4t")
          w6t_sb = cload(w6t, (HID, OUT), "w6t")
          b01_sb = cload(b01, (1, HID), "b01")
          b46_sb = cload(b46, (1, OUT), "b46")
          selft_sb = cload(self_t, (FEAT, bc), "selft")
          oht_sb = cload(one_hop_t, (FEAT, g), "oht")
          ohn_sb = cload(one_hop_n, (128, nt1 * FEAT), "ohn")

          # bd15 masks for the layer-2 (soft1) segment sums (DVE)
          bd15s = []
          for k in range(nt1 // 5):
              bd15 = const.tile([128, 40], F16, tag=f"bd15_{k}")
              nc.vector.tensor_mul(
                  bd15[:].rearrange("p (j m) -> p j m", m=8),
                  mask_sb[:].rearrange("p (j m) -> p j m", m=8),
                  e1f_sb[:, 5 * k:5 * k + 5].to_broadcast([128, 5, 8]),
              )
              bd15s.append(bd15)

          rz2rep_sb = sbig.tile([128, g], F32, tag="rz2rep")
          rz1rep_sb = sbig.tile([128, bc], F32, tag="rz1rep")
          agg2t_sb = sbig.tile([128, g], F16, tag="agg2t")
          xos_sb = sbig.tile([128, g], F16, tag="xos")
          yt_sb = sbig.tile([128, bc], F16, tag="yt")
          a1t_sb = sbig.tile([128, bc], F16, tag="a1t")
          py = p_acc.tile([128, bc], F32, tag="py")
          pa1 = p_acc.tile([128, bc], F32, tag="pa1")

          def emit_pre_tail():
              """PE/DVE setup work emitted alongside supertile 0's stream MMs:
              replicated 1/Z rows, and the py/pa1 zero-fills."""
              for s in range(nst):
                  cols = ST_COLS
                  rp = p_misc.tile([128, cols], F32, tag="misc")
                  nc.tensor.matmul(
                      rp[:], ones_row[:1, :128],
                      rz2row[:1, cols * s:cols * (s + 1)],
                      start=True, stop=True, skip_group_check=True,
                  )
                  nc.vector.tensor_copy(rz2rep_sb[:, cols * s:cols * (s + 1)], rp[:])
              rp = p_misc.tile([128, bc], F32, tag="misc")
              nc.tensor.matmul(rp[:], ones_row[:1, :128], rz1row[:1, :],
                               start=True, stop=True, skip_group_check=True)
              nc.vector.tensor_copy(rz1rep_sb[:], rp[:])
              nc.tensor.matmul(py[:], ones_row[:1, :128], zeros_row[:1, :bc],
                               start=True, stop=False, skip_group_check=True)
              nc.tensor.matmul(pa1[:], ones_row[:1, :128], zeros_row[:1, :bc],
                               start=True, stop=False, skip_group_check=True)

          def emit_tail(sp):
              """Phase-2 (x_one_s) + layer-2 masked matmuls for supertile sp;
              emitted while supertile sp+1 streams."""
              for ci in range(xcs):
                  c = xcs * sp + ci
                  p2 = p_misc.tile([128, HID], F32, tag="misc")
                  nc.tensor.matmul(
                      p2[:], ones_row[:1, :128], b01_sb[:1, :],
                      start=True, stop=False, skip_group_check=True,
                  )
                  nc.tensor.matmul(
                      p2[:], oht_sb[:, 128 * c:128 * (c + 1)], w0t_sb[:],
                      start=False, stop=False, skip_group_check=True,
                  )
                  nc.tensor.matmul(
                      p2[:], agg2t_sb[:, 128 * c:128 * (c + 1)], w2t_sb[:],
                      start=False, stop=True, skip_group_check=True,
                  )
                  nc.scalar.activation(
                      xos_sb[:, 128 * c:128 * (c + 1)], p2[:], AF.Relu)
              for ci in range(xcs):
                  t = xcs * sp + ci
                  w = _phase_width((128 * t) % 20)
                  bf = (128 * t) // 20
                  bd = bd15s[t // 5]
                  jj = t % 5
                  nc.tensor.matmul(
                      py[:, bf:bf + w], xos_sb[:, 128 * t:128 * (t + 1)],
                      bd[:, 8 * jj:8 * jj + w],
                      start=False, stop=(t == nt1 - 1), skip_group_check=True,
                  )
                  nc.tensor.matmul(
                      pa1[:, bf:bf + w], ohn_sb[:, 128 * t:128 * (t + 1)],
                      bd[:, 8 * jj:8 * jj + w],
                      start=False, stop=(t == nt1 - 1), skip_group_check=True,
                  )

          # ---- phase 1: agg2T[f, group], streamed --------------------------
          def emit_evict(s, pag_s):
              """Normalize + evict supertile s's PSUM accumulator (DVE)."""
              nc.vector.tensor_mul(
                  agg2t_sb[:, ST_COLS * s:ST_COLS * (s + 1)], pag_s[:],
                  rz2rep_sb[:, ST_COLS * s:ST_COLS * (s + 1)],
              )

          pag = None
          pending_evict = None
          for c in range(nch):
              xt = xpool.tile([128, CH * FEAT], F16, tag="x")
              if mode != "nodma":
                  nc.sync.dma_start(xt[:], two_hop[:, CH * FEAT * c:CH * FEAT * (c + 1)])
              # bd tiles for the whole chunk first: the DVE FIFO must hand the
              # PE its masks before blocking on eviction/copy work below.
              bds = []
              for h5 in range(CH // 5):
                  tg0 = CH * c + 5 * h5          # global tile idx of this 5-group
                  tl0 = tg0 % TPS                # tile idx within supertile
                  s = tg0 // TPS                 # supertile idx
                  if tl0 == 0:
                      pag = p_agg.tile([128, ST_COLS], F32, tag="agg")
                      nc.tensor.matmul(
                          pag[:], ones_row[:1, :128], zeros_row[:1, :ST_COLS],
                          start=True, stop=False, skip_group_check=True,
                      )
                  bd5 = bdpool.tile([128, 40], F16, tag="bd5")
                  nc.vector.tensor_mul(
                      bd5[:].rearrange("p (j m) -> p j m", m=8),
                      mask_sb[:].rearrange("p (j m) -> p j m", m=8),
                      eflat_sb[:, tg0:tg0 + 5].to_broadcast([128, 5, 8]),
                  )
                  bds.append(bd5)
              if mode == "dmaonly":
                  continue
              if pending_evict is not None:
                  emit_evict(*pending_evict)
                  pending_evict = None
              if c == cps - 1:
                  emit_pre_tail()
              for h5 in range(CH // 5):
                  tg0 = CH * c + 5 * h5
                  tl0 = tg0 % TPS
                  s = tg0 // TPS
                  for j in range(5):
                      tl = tl0 + j
                      w = _phase_width((128 * tl) % 20)
                      gf = (128 * tl) // 20
                      nc.tensor.matmul(
                          pag[:, gf:gf + w],
                          xt[:, FEAT * (5 * h5 + j):FEAT * (5 * h5 + j + 1)],
                          bds[h5][:, 8 * j:8 * j + w],
                          start=False, stop=(tl == TPS - 1), skip_group_check=True,
                      )
                  if tl0 + 5 == TPS:
                      pending_evict = (s, pag)
              # interleave dependent tail work while the next chunks stream
              if c % cps == 0 and c > 0:
                  emit_tail(c // cps - 1)        # supertile just finished
          if mode == "dmaonly":
              continue
          emit_evict(*pending_evict)
          pending_evict = None
          emit_tail(nst - 1)

          # ---- y/a1 evictions (1/Z1), x_s_one, final layer -----------------
          nc.vector.tensor_mul(yt_sb[:], py[:], rz1rep_sb[:])
          nc.vector.tensor_mul(a1t_sb[:], pa1[:], rz1rep_sb[:])

          pxs = p_acc.tile([128, bc], F32, tag="pxs")
          nc.tensor.matmul(pxs[:], b01_sb[:1, :], ones_row[:1, :bc],
                           start=True, stop=False, skip_group_check=True)
          nc.tensor.matmul(pxs[:], w0t_sb[:], selft_sb[:],
                           start=False, stop=False, skip_group_check=True)
          nc.tensor.matmul(pxs[:], w2t_sb[:], a1t_sb[:],
                           start=False, stop=True, skip_group_check=True)
          xst_sb = sbig.tile([128, bc], F16, tag="xst")
          nc.scalar.activation(xst_sb[:], pxs[:], AF.Relu)

          for j in range(nbt):
              po = p_misc.tile([128, OUT], F32, tag="misc")
              nc.tensor.matmul(po[:], ones_row[:1, :128], b46_sb[:1, :],
                               start=True, stop=False, skip_group_check=True)
              nc.tensor.matmul(po[:], xst_sb[:, 128 * j:128 * (j + 1)], w4t_sb[:],
                               start=False, stop=False, skip_group_check=True)
              nc.tensor.matmul(po[:], yt_sb[:, 128 * j:128 * (j + 1)], w6t_sb[:],
                               start=False, stop=True, skip_group_check=True)
              ot = spool.tile([128, OUT], F32, tag="ot")
              nc.scalar.copy(ot[:], po[:])
              nc.sync.dma_start(out_d[128 * j:128 * (j + 1), :], ot[:])

    nc.compile()
    return nc


def make_in_maps(inputs: dict, bc: int = BC, ncores: int = NCORES):
    """Host-side shard + auxiliary layout prep. Returns list of per-core dicts."""
    f32 = np.float32
    f16 = np.float16
    self_feat = np.asarray(inputs["self_feat"], f32)
    one_hop = np.asarray(inputs["one_hop_feat"], f32)
    two_hop = np.asarray(inputs["two_hop_feat"], f32)
    e_time = np.asarray(inputs["e_time"], f32)
    his_time = np.asarray(inputs["his_time"], f32)
    his_his = np.asarray(inputs["his_his_time"], f32)
    W0 = np.asarray(inputs["W0"], f32)
    b0 = np.asarray(inputs["b0"], f32)
    W2 = np.asarray(inputs["W2"], f32)
    b2 = np.asarray(inputs["b2"], f32)
    W4 = np.asarray(inputs["W4"], f32)
    b4 = np.asarray(inputs["b4"], f32)
    W6 = np.asarray(inputs["W6"], f32)
    b6 = np.asarray(inputs["b6"], f32)
    delta = float(np.asarray(inputs["delta"]).reshape(-1)[0])

    g = bc * HIST
    r2 = g * HIST

    def tiled(x):
        """[N*128, 128] -> [128, N*128] with row 128t+p at [p, 128t+f]."""
        n = x.shape[0] // 128
        return np.ascontiguousarray(
            x.reshape(n, 128, FEAT).transpose(1, 0, 2).reshape(128, n * FEAT)
        )

    C = np.ascontiguousarray
    shared = {
        "w0t": C(W0.T).astype(f16), "w2t": C(W2.T).astype(f16),
        "w4t": C(W4.T).astype(f16), "w6t": C(W6.T).astype(f16),
        "b01": (b0 + b2).reshape(1, HID).astype(f16),
        "b46": (b4 + b6).reshape(1, OUT).astype(f16),
        "bdmask": build_bdmask(),
    }
    maps = []
    for c in range(ncores):
        bs = slice(c * bc, (c + 1) * bc)
        oh = one_hop[c * g:(c + 1) * g]
        l1 = delta * (his_time[bs] - e_time[bs, None])    # [bc, H]
        l2 = delta * (his_his[bs] - his_time[bs, :, None])   # [bc, H, H]
        maps.append({
            "two_hop": tiled(two_hop[c * r2:(c + 1) * r2].astype(f16)),
            "one_hop_n": tiled(oh.astype(f16)),
            "one_hop_t": C(oh.T).astype(f16),
            "self_t": C(self_feat[bs].T).astype(f16),
            "l1f": C(l1.reshape(g // 128, 128).T).astype(f16),
            "l1n": C(l1),
            "l2n": C(l2.reshape(bc, HIST * HIST)).astype(f16),
            "l2f": C(l2.reshape(r2 // 128, 128).T).astype(f16),
            **shared,
        })
    return maps


def kernel(**inputs) -> np.ndarray:
    from concourse.bass_utils import run_bass_kernel_spmd

    nc = build_program(BC)
    in_maps = make_in_maps(inputs)
    res = run_bass_kernel_spmd(nc, in_maps, core_ids=list(range(NCORES)))
    return np.concatenate([res.results[c]["out"] for c in range(NCORES)], axis=0)


# revision 21
# speedup vs baseline: 5.4369x; 1.0202x over previous
"""Trainium2 Bass kernel for the DGNN message-passing module (fp16 rev2).

Contract: kernel(**inputs) takes the FULL unsharded inputs and returns
the full [2048, 64] float32 output.  Internally the leading B (event)
dimension is sharded across 8 NeuronCores (pure data parallel); small
weights are replicated.

Math (per core, b=256, H=20, FEAT=HID=128, OUT=64):
  soft1 = softmax(-delta*(e_time[:,None]-his_time), axis=1)
  soft2 = softmax(-delta*(his_time[:,:,None]-his_his_time), axis=2)
  agg1[b]   = sum_h soft1[b,h] * one_hop[b,h,:]          (linearity pull-out)
  agg2[b,h] = sum_k soft2[b,h,k] * two_hop[b,h,k,:]
  x_s_one = relu(self@W0.T + agg1@W2.T + b0+b2)
  x_one_s = relu(one_hop@W0.T + agg2@W2.T + b0+b2)
  y[b]    = sum_h soft1[b,h] * x_one_s[b,h,:]
  out     = x_s_one@W4.T + y@W6.T + b4+b6

Strategy: stream two_hop as fp16 in a [128, 800*128] row-tiled layout
(row 128t+p at [p, 128t+f]) in 20-tile chunks on a dedicated DMA issue
queue (Sync engine); everything else DMAs on the Activation engine with
the bd-weight inputs (l2f) kicked first so the tensor engine starts
within ~2us of the first chunk landing.  agg2 is a tensor-engine
weighted segment-sum: per 128-row tile (fp16 stationary, FWL) a
[128, <=8] "block diagonal" mask*exp(logit) moving tile accumulates
group columns in PSUM; softmax 1/Z2 is folded into the PSUM eviction
(GpSimd multiply by a replicated 1/Z row).  soft1 is likewise applied
unnormalized via exp(l1) masked matmuls with 1/Z1 folded into the y/a1
evictions.  Phase-2 (x_one_s) and the layer-2 aggregations are
interleaved into the stream at supertile boundaries so the PE tail
after the last DMA is only the final ~2us.
"""

import sys

import numpy as np

sys.path.insert(0, "/opt/trn_rl_repo")

B, HIST, FEAT, HID, OUT = 2048, 20, 128, 128, 64
NCORES = 8
BC = B // NCORES          # 256 events per core
G = BC * HIST             # 5120 (b,h) groups per core
R2 = G * HIST             # 102400 two-hop rows per core
NT2 = R2 // 128           # 800 two_hop row-tiles
NT1 = G // 128            # 40 one_hop row-tiles
ST_COLS = 512             # PSUM group-columns per supertile (1 bank of fp32)
TPS = ST_COLS * HIST // 128   # 80 two_hop tiles per supertile
CH = 20                   # two_hop tiles per DMA chunk (4 bd-builds each)

# (128*t) % 20 cycles with period 5; per-phase mask width (# groups touched
# by a 128-row pass).
PHIS = [0, 8, 16, 4, 12]


def _phase_width(phi: int) -> int:
    return (phi + 127) // 20 + 1


def build_bdmask() -> np.ndarray:
    """[128, 40] = 5 masks of [128, 8]: mask[p, 8*i + m] = 1 if (phi_i+p)//20 == m."""
    m = np.zeros((128, 40), np.float16)
    for i, phi in enumerate(PHIS):
        for p in range(128):
            m[p, 8 * i + (phi + p) // 20] = 1.0
    return m


def build_program(bc: int = BC, repeat: int = 1, mode: str = "full"):
    """Build the SPMD Bass program (one NeuronCore's view). Returns nc.

    repeat>1 duplicates the whole compute body (timing harness only).
    mode: "full" | "dmaonly" (stream two_hop, skip phase-1 matmuls) |
    "nodma" (skip the two_hop stream DMAs)."""
    import concourse.bass as bass
    import concourse.tile as tile
    from concourse import bacc, mybir
    from contextlib import ExitStack

    F32 = mybir.dt.float32
    F16 = mybir.dt.float16
    AF = mybir.ActivationFunctionType
    g = bc * HIST
    r2 = g * HIST
    nbt = bc // 128              # b-chunks (2)
    nt1 = g // 128               # 128-row passes over one_hop / x_one_s (40)
    nt2 = r2 // 128
    nst = (g + ST_COLS - 1) // ST_COLS   # supertiles (10)
    nch = nt2 // CH              # two_hop DMA chunks (40)
    cps = TPS // CH              # chunks per supertile (4)
    xcs = ST_COLS // 128         # xos chunks per supertile (4)

    nc = bacc.Bacc("TRN2", target_bir_lowering=False, debug=False)

    def din(name, shape, dt=F16):
        return nc.dram_tensor(name, list(shape), dt, kind="ExternalInput").ap()

    two_hop = din("two_hop", (128, nt2 * FEAT))       # tiled [p, 128t+f]
    one_hop_n = din("one_hop_n", (128, nt1 * FEAT))   # tiled natural
    one_hop_t = din("one_hop_t", (FEAT, g))           # transposed
    self_t = din("self_t", (FEAT, bc))
    l1f = din("l1f", (128, nt1))          # soft1 logits flat-tiled [p,t]=v[128t+p]
    l1n = din("l1n", (bc, HIST), F32)     # soft1 logits natural (for Z1)
    l2n = din("l2n", (bc, HIST * HIST))   # delta*(his_his - his_time[:,:,None])
    l2f = din("l2f", (128, nt2))          # same, flat-transposed [p, t] = v[128t+p]
    w0t = din("w0t", (FEAT, HID))
    w2t = din("w2t", (FEAT, HID))
    w4t = din("w4t", (HID, OUT))
    w6t = din("w6t", (HID, OUT))
    b01 = din("b01", (1, HID))
    b46 = din("b46", (1, OUT))
    bdmask = din("bdmask", (128, 40))
    out_d = nc.dram_tensor("out", [bc, OUT], F32, kind="ExternalOutput").ap()

    with tile.TileContext(nc) as tc, ExitStack() as ctx:
        const = ctx.enter_context(tc.tile_pool(name="const", bufs=1))
        sbig = ctx.enter_context(tc.tile_pool(name="sbig", bufs=1))
        xpool = ctx.enter_context(tc.tile_pool(name="xp", bufs=6))
        bdpool = ctx.enter_context(tc.tile_pool(name="bdp", bufs=8))
        spool = ctx.enter_context(tc.tile_pool(name="sp", bufs=4))
        dpool = ctx.enter_context(tc.tile_pool(name="dram", bufs=1, space="DRAM"))
        p_agg = ctx.enter_context(tc.tile_pool(name="pagg", bufs=2, space="PSUM"))
        p_misc = ctx.enter_context(tc.tile_pool(name="pmisc", bufs=2, space="PSUM"))
        p_acc = ctx.enter_context(tc.tile_pool(name="pacc", bufs=1, space="PSUM"))

        # ---- priority DMAs (Activation engine queue): bd-weight inputs ----
        l2f_sb = const.tile([128, nt2], F16, tag="l2f")
        nc.scalar.dma_start(l2f_sb[:], l2f)
        mask_sb = const.tile([128, 40], F16, tag="mask")
        nc.scalar.dma_start(mask_sb[:], bdmask)
        l1f_sb = const.tile([128, nt1], F16, tag="l1f")
        nc.scalar.dma_start(l1f_sb[:], l1f)

        # e_flat = exp(l2f): unnormalized soft2 weight for row 128t+p at [p,t]
        eflat_sb = const.tile([128, nt2], F16, tag="eflat")
        nc.scalar.activation(eflat_sb[:], l2f_sb[:], AF.Exp)
        # e1f = exp(l1f): unnormalized soft1 weight, flat-tiled
        e1f_sb = const.tile([128, nt1], F16, tag="e1f")
        nc.scalar.activation(e1f_sb[:], l1f_sb[:], AF.Exp)

        l1ts, l2ts = [], []
        for j in range(nbt):
            l1t = spool.tile([128, HIST], F32, tag=f"l1_{j}")
            nc.scalar.dma_start(l1t[:], l1n[128 * j:128 * (j + 1), :])
            l1ts.append(l1t)
            l2t = spool.tile([128, HIST * HIST], F16, tag=f"l2_{j}")
            nc.scalar.dma_start(l2t[:], l2n[128 * j:128 * (j + 1), :])
            l2ts.append(l2t)

        ones_row = const.tile([1, ST_COLS], F16, tag="ones")
        zeros_row = const.tile([1, ST_COLS], F16, tag="zeros")
        nc.vector.memset(ones_row[:], 1.0)
        nc.vector.memset(zeros_row[:], 0.0)

        # (body below may be repeated for the timing harness)
        for _rep in range(repeat):
          # ---- softmax normalizers (flat rows via tiny DRAM round-trip) ---
          d_rz1 = dpool.tile([bc, 1], F16, tag="drz1")
          d_rz2 = dpool.tile([bc, HIST], F16, tag="drz2")
          for j in range(nbt):
              e1 = spool.tile([128, HIST], F32, tag="e1")
              nc.scalar.activation(e1[:], l1ts[j][:], AF.Exp)
              z1 = spool.tile([128, 1], F32, tag="z1")
              nc.vector.reduce_sum(z1[:], e1[:], axis=mybir.AxisListType.X)
              rz1 = spool.tile([128, 1], F16, tag="rz1")
              with nc.allow_low_precision(reason="1/Z in fp16: 5e-4 rel, gate is 2e-2"):
                  nc.vector.reciprocal(rz1[:], z1[:])
              nc.gpsimd.dma_start(d_rz1[128 * j:128 * (j + 1), :], rz1[:])

              e2 = spool.tile([128, HIST * HIST], F32, tag="e2")
              nc.scalar.activation(e2[:], l2ts[j][:], AF.Exp)
              z2 = spool.tile([128, HIST], F32, tag="z2")
              nc.vector.reduce_sum(
                  z2[:],
                  e2[:].rearrange("p (h k) -> p h k", k=HIST),
                  axis=mybir.AxisListType.X,
              )
              rz2 = spool.tile([128, HIST], F16, tag="rz2")
              with nc.allow_low_precision(reason="1/Z in fp16: 5e-4 rel, gate is 2e-2"):
                  nc.vector.reciprocal(rz2[:], z2[:])
              nc.gpsimd.dma_start(d_rz2[128 * j:128 * (j + 1), :], rz2[:])

          rz1row = const.tile([1, bc], F16, tag="rz1row")
          nc.gpsimd.dma_start(rz1row[:1, :], d_rz1[:].rearrange("a b -> (a b)"))
          rz2row = const.tile([1, g], F16, tag="rz2row")
          nc.gpsimd.dma_start(rz2row[:1, :], d_rz2[:].rearrange("a b -> (a b)"))

          # ---- remaining constants (Activation engine queue) --------------
          def cload(ap, shape, tag, dt=F16):
              t = const.tile(list(shape), dt, tag=tag)
              nc.scalar.dma_start(t[:], ap)
              return t

          w0t_sb = cload(w0t, (FEAT, HID), "w0t")
          w2t_sb = cload(w2t, (FEAT, HID), "w2t")
          w4t_sb = cload(w4t, (HID, OUT), "w4t")
          w6t_sb = cload(w6t, (HID, OUT), "w6t")
          b01_sb = cload(b01, (1, HID), "b01")
          b46_sb = cload(b46, (1, OUT), "b46")
          selft_sb = cload(self_t, (FEAT, bc), "selft")
          oht_sb = cload(one_hop_t, (FEAT, g), "oht")
          ohn_sb = cload(one_hop_n, (128, nt1 * FEAT), "ohn")

          # bd15 masks for the layer-2 (soft1) segment sums (DVE)
          bd15s = []
          for k in range(nt1 // 5):
              bd15 = const.tile([128, 40], F16, tag=f"bd15_{k}")
              nc.vector.tensor_mul(
                  bd15[:].rearrange("p (j m) -> p j m", m=8),
                  mask_sb[:].rearrange("p (j m) -> p j m", m=8),
                  e1f_sb[:, 5 * k:5 * k + 5].to_broadcast([128, 5, 8]),
              )
              bd15s.append(bd15)

          rz2rep_sb = sbig.tile([128, g], F32, tag="rz2rep")
          rz1rep_sb = sbig.tile([128, bc], F32, tag="rz1rep")
          agg2t_sb = sbig.tile([128, g], F16, tag="agg2t")
          xos_sb = sbig.tile([128, g], F16, tag="xos")
          yt_sb = sbig.tile([128, bc], F16, tag="yt")
          a1t_sb = sbig.tile([128, bc], F16, tag="a1t")
          py = p_acc.tile([128, bc], F32, tag="py")
          pa1 = p_acc.tile([128, bc], F32, tag="pa1")

          def emit_pre_tail():
              """PE/DVE setup work emitted alongside supertile 0's stream MMs:
              replicated 1/Z rows, and the py/pa1 zero-fills."""
              for s in range(nst):
                  cols = ST_COLS
                  rp = p_misc.tile([128, cols], F32, tag="misc")
                  nc.tensor.matmul(
                      rp[:], ones_row[:1, :128],
                      rz2row[:1, cols * s:cols * (s + 1)],
                      start=True, stop=True, skip_group_check=True,
                  )
                  nc.vector.tensor_copy(rz2rep_sb[:, cols * s:cols * (s + 1)], rp[:])
              rp = p_misc.tile([128, bc], F32, tag="misc")
              nc.tensor.matmul(rp[:], ones_row[:1, :128], rz1row[:1, :],
                               start=True, stop=True, skip_group_check=True)
              nc.vector.tensor_copy(rz1rep_sb[:], rp[:])
              nc.tensor.matmul(py[:], ones_row[:1, :128], zeros_row[:1, :bc],
                               start=True, stop=False, skip_group_check=True)
              nc.tensor.matmul(pa1[:], ones_row[:1, :128], zeros_row[:1, :bc],
                               start=True, stop=False, skip_group_check=True)

          def emit_tail(sp):
              """Phase-2 (x_one_s) + layer-2 masked matmuls for supertile sp;
              emitted while supertile sp+1 streams."""
              for ci in range(xcs):
                  c = xcs * sp + ci
                  p2 = p_misc.tile([128, HID], F32, tag="misc")
                  nc.tensor.matmul(
                      p2[:], ones_row[:1, :128], b01_sb[:1, :],
                      start=True, stop=False, skip_group_check=True,
                  )
                  nc.tensor.matmul(
                      p2[:], oht_sb[:, 128 * c:128 * (c + 1)], w0t_sb[:],
                      start=False, stop=False, skip_group_check=True,
                  )
                  nc.tensor.matmul(
                      p2[:], agg2t_sb[:, 128 * c:128 * (c + 1)], w2t_sb[:],
                      start=False, stop=True, skip_group_check=True,
                  )
                  nc.scalar.activation(
                      xos_sb[:, 128 * c:128 * (c + 1)], p2[:], AF.Relu)
              for ci in range(xcs):
                  t = xcs * sp + ci
                  w = _phase_width((128 * t) % 20)
                  bf = (128 * t) // 20
                  bd = bd15s[t // 5]
                  jj = t % 5
                  nc.tensor.matmul(
                      py[:, bf:bf + w], xos_sb[:, 128 * t:128 * (t + 1)],
                      bd[:, 8 * jj:8 * jj + w],
                      start=False, stop=(t == nt1 - 1), skip_group_check=True,
                  )
                  nc.tensor.matmul(
                      pa1[:, bf:bf + w], ohn_sb[:, 128 * t:128 * (t + 1)],
                      bd[:, 8 * jj:8 * jj + w],
                      start=False, stop=(t == nt1 - 1), skip_group_check=True,
                  )

          # ---- phase 1: agg2T[f, group], streamed --------------------------
          def emit_evict(s, pag_s):
              """Normalize + evict supertile s's PSUM accumulator (DVE)."""
              nc.vector.tensor_mul(
                  agg2t_sb[:, ST_COLS * s:ST_COLS * (s + 1)], pag_s[:],
                  rz2rep_sb[:, ST_COLS * s:ST_COLS * (s + 1)],
              )

          pag = None
          pending_evict = None
          for c in range(nch):
              xt = xpool.tile([128, CH * FEAT], F16, tag="x")
              if mode != "nodma":
                  nc.sync.dma_start(xt[:], two_hop[:, CH * FEAT * c:CH * FEAT * (c + 1)])
              # bd tiles for the whole chunk first: the DVE FIFO must hand the
              # PE its masks before blocking on eviction/copy work below.
              bds = []
              for h5 in range(CH // 5):
                  tg0 = CH * c + 5 * h5          # global tile idx of this 5-group
                  tl0 = tg0 % TPS                # tile idx within supertile
                  s = tg0 // TPS                 # supertile idx
                  if tl0 == 0:
                      pag = p_agg.tile([128, ST_COLS], F32, tag="agg")
                      nc.tensor.matmul(
                          pag[:], ones_row[:1, :128], zeros_row[:1, :ST_COLS],
                          start=True, stop=False, skip_group_check=True,
                      )
                  bd5 = bdpool.tile([128, 40], F16, tag="bd5")
                  nc.vector.tensor_mul(
                      bd5[:].rearrange("p (j m) -> p j m", m=8),
                      mask_sb[:].rearrange("p (j m) -> p j m", m=8),
                      eflat_sb[:, tg0:tg0 + 5].to_broadcast([128, 5, 8]),
                  )
                  bds.append(bd5)
              if mode == "dmaonly":
                  continue
              if c == cps:
                  # before the first eviction: DVE program order must have the
                  # rz2rep copies ahead of evict(s0)
                  emit_pre_tail()
              if pending_evict is not None:
                  emit_evict(*pending_evict)
                  pending_evict = None
              for h5 in range(CH // 5):
                  tg0 = CH * c + 5 * h5
                  tl0 = tg0 % TPS
                  s = tg0 // TPS
                  for j in range(5):
                      tl = tl0 + j
                      w = _phase_width((128 * tl) % 20)
                      gf = (128 * tl) // 20
                      nc.tensor.matmul(
                          pag[:, gf:gf + w],
                          xt[:, FEAT * (5 * h5 + j):FEAT * (5 * h5 + j + 1)],
                          bds[h5][:, 8 * j:8 * j + w],
                          start=False, stop=(tl == TPS - 1), skip_group_check=True,
                      )
                  if tl0 + 5 == TPS:
                      pending_evict = (s, pag)
              # interleave dependent tail work while the next chunks stream
              if c % cps == 1 and c > cps:
                  emit_tail(c // cps - 1)        # supertile finished a chunk ago
          if mode == "dmaonly":
              continue
          emit_evict(*pending_evict)
          pending_evict = None
          emit_tail(nst - 1)

          # ---- y/a1 evictions (1/Z1), x_s_one, final layer -----------------
          nc.vector.tensor_mul(yt_sb[:], py[:], rz1rep_sb[:])
          nc.vector.tensor_mul(a1t_sb[:], pa1[:], rz1rep_sb[:])

          pxs = p_acc.tile([128, bc], F32, tag="pxs")
          nc.tensor.matmul(pxs[:], b01_sb[:1, :], ones_row[:1, :bc],
                           start=True, stop=False, skip_group_check=True)
          nc.tensor.matmul(pxs[:], w0t_sb[:], selft_sb[:],
                           start=False, stop=False, skip_group_check=True)
          nc.tensor.matmul(pxs[:], w2t_sb[:], a1t_sb[:],
                           start=False, stop=True, skip_group_check=True)
          xst_sb = sbig.tile([128, bc], F16, tag="xst")
          nc.scalar.activation(xst_sb[:], pxs[:], AF.Relu)

          for j in range(nbt):
              po = p_misc.tile([128, OUT], F32, tag="misc")
              nc.tensor.matmul(po[:], ones_row[:1, :128], b46_sb[:1, :],
                               start=True, stop=False, skip_group_check=True)
              nc.tensor.matmul(po[:], xst_sb[:, 128 * j:128 * (j + 1)], w4t_sb[:],
                               start=False, stop=False, skip_group_check=True)
              nc.tensor.matmul(po[:], yt_sb[:, 128 * j:128 * (j + 1)], w6t_sb[:],
                               start=False, stop=True, skip_group_check=True)
              ot = spool.tile([128, OUT], F32, tag="ot")
              nc.scalar.copy(ot[:], po[:])
              nc.sync.dma_start(out_d[128 * j:128 * (j + 1), :], ot[:])

    nc.compile()
    return nc


def make_in_maps(inputs: dict, bc: int = BC, ncores: int = NCORES):
    """Host-side shard + auxiliary layout prep. Returns list of per-core dicts."""
    f32 = np.float32
    f16 = np.float16
    self_feat = np.asarray(inputs["self_feat"], f32)
    one_hop = np.asarray(inputs["one_hop_feat"], f32)
    two_hop = np.asarray(inputs["two_hop_feat"], f32)
    e_time = np.asarray(inputs["e_time"], f32)
    his_time = np.asarray(inputs["his_time"], f32)
    his_his = np.asarray(inputs["his_his_time"], f32)
    W0 = np.asarray(inputs["W0"], f32)
    b0 = np.asarray(inputs["b0"], f32)
    W2 = np.asarray(inputs["W2"], f32)
    b2 = np.asarray(inputs["b2"], f32)
    W4 = np.asarray(inputs["W4"], f32)
    b4 = np.asarray(inputs["b4"], f32)
    W6 = np.asarray(inputs["W6"], f32)
    b6 = np.asarray(inputs["b6"], f32)
    delta = float(np.asarray(inputs["delta"]).reshape(-1)[0])

    g = bc * HIST
    r2 = g * HIST

    def tiled(x):
        """[N*128, 128] -> [128, N*128] with row 128t+p at [p, 128t+f]."""
        n = x.shape[0] // 128
        return np.ascontiguousarray(
            x.reshape(n, 128, FEAT).transpose(1, 0, 2).reshape(128, n * FEAT)
        )

    C = np.ascontiguousarray
    shared = {
        "w0t": C(W0.T).astype(f16), "w2t": C(W2.T).astype(f16),
        "w4t": C(W4.T).astype(f16), "w6t": C(W6.T).astype(f16),
        "b01": (b0 + b2).reshape(1, HID).astype(f16),
        "b46": (b4 + b6).reshape(1, OUT).astype(f16),
        "bdmask": build_bdmask(),
    }
    maps = []
    for c in range(ncores):
        bs = slice(c * bc, (c + 1) * bc)
        oh = one_hop[c * g:(c + 1) * g]
        l1 = delta * (his_time[bs] - e_time[bs, None])    # [bc, H]
        l2 = delta * (his_his[bs] - his_time[bs, :, None])   # [bc, H, H]
        maps.append({
            "two_hop": tiled(two_hop[c * r2:(c + 1) * r2].astype(f16)),
            "one_hop_n": tiled(oh.astype(f16)),
            "one_hop_t": C(oh.T).astype(f16),
            "self_t": C(self_feat[bs].T).astype(f16),
            "l1f": C(l1.reshape(g // 128, 128).T).astype(f16),
            "l1n": C(l1),
            "l2n": C(l2.reshape(bc, HIST * HIST)).astype(f16),
            "l2f": C(l2.reshape(r2 // 128, 128).T).astype(f16),
            **shared,
        })
    return maps


def kernel(**inputs) -> np.ndarray:
    from concourse.bass_utils import run_bass_kernel_spmd

    nc = build_program(BC)
    in_maps = make_in_maps(inputs)
    res = run_bass_kernel_spmd(nc, in_maps, core_ids=list(range(NCORES)))
    return np.concatenate([res.results[c]["out"] for c in range(NCORES)], axis=0)


# revision 22
# speedup vs baseline: 5.8579x; 1.0774x over previous
"""Trainium2 Bass kernel for the DGNN message-passing module (fp16 rev2).

Contract: kernel(**inputs) takes the FULL unsharded inputs and returns
the full [2048, 64] float32 output.  Internally the leading B (event)
dimension is sharded across 8 NeuronCores (pure data parallel); small
weights are replicated.

Math (per core, b=256, H=20, FEAT=HID=128, OUT=64):
  soft1 = softmax(-delta*(e_time[:,None]-his_time), axis=1)
  soft2 = softmax(-delta*(his_time[:,:,None]-his_his_time), axis=2)
  agg1[b]   = sum_h soft1[b,h] * one_hop[b,h,:]          (linearity pull-out)
  agg2[b,h] = sum_k soft2[b,h,k] * two_hop[b,h,k,:]
  x_s_one = relu(self@W0.T + agg1@W2.T + b0+b2)
  x_one_s = relu(one_hop@W0.T + agg2@W2.T + b0+b2)
  y[b]    = sum_h soft1[b,h] * x_one_s[b,h,:]
  out     = x_s_one@W4.T + y@W6.T + b4+b6

Strategy: stream two_hop as fp16 in a [128, 800*128] row-tiled layout
(row 128t+p at [p, 128t+f]) in 20-tile chunks on a dedicated DMA issue
queue (Sync engine); everything else DMAs on the Activation engine with
the bd-weight inputs (l2f) kicked first so the tensor engine starts
within ~2us of the first chunk landing.  agg2 is a tensor-engine
weighted segment-sum: per 128-row tile (fp16 stationary, FWL) a
[128, <=8] "block diagonal" mask*exp(logit) moving tile accumulates
group columns in PSUM; softmax 1/Z2 is folded into the PSUM eviction
(GpSimd multiply by a replicated 1/Z row).  soft1 is likewise applied
unnormalized via exp(l1) masked matmuls with 1/Z1 folded into the y/a1
evictions.  Phase-2 (x_one_s) and the layer-2 aggregations are
interleaved into the stream at supertile boundaries so the PE tail
after the last DMA is only the final ~2us.
"""

import sys

import numpy as np

sys.path.insert(0, "/opt/trn_rl_repo")

B, HIST, FEAT, HID, OUT = 2048, 20, 128, 128, 64
NCORES = 8
BC = B // NCORES          # 256 events per core
G = BC * HIST             # 5120 (b,h) groups per core
R2 = G * HIST             # 102400 two-hop rows per core
NT2 = R2 // 128           # 800 two_hop row-tiles
NT1 = G // 128            # 40 one_hop row-tiles
ST_COLS = 512             # PSUM group-columns per supertile (1 bank of fp32)
TPS = ST_COLS * HIST // 128   # 80 two_hop tiles per supertile
CH = 20                   # two_hop tiles per DMA chunk (4 bd-builds each)

# (128*t) % 20 cycles with period 5; per-phase mask width (# groups touched
# by a 128-row pass).
PHIS = [0, 8, 16, 4, 12]


def _phase_width(phi: int) -> int:
    return (phi + 127) // 20 + 1


def build_bdmask() -> np.ndarray:
    """[128, 40] = 5 masks of [128, 8]: mask[p, 8*i + m] = 1 if (phi_i+p)//20 == m."""
    m = np.zeros((128, 40), np.float16)
    for i, phi in enumerate(PHIS):
        for p in range(128):
            m[p, 8 * i + (phi + p) // 20] = 1.0
    return m


def build_program(bc: int = BC, repeat: int = 1, mode: str = "full"):
    """Build the SPMD Bass program (one NeuronCore's view). Returns nc.

    repeat>1 duplicates the whole compute body (timing harness only).
    mode: "full" | "dmaonly" (stream two_hop, skip phase-1 matmuls) |
    "nodma" (skip the two_hop stream DMAs)."""
    import concourse.bass as bass
    import concourse.tile as tile
    from concourse import bacc, mybir
    from contextlib import ExitStack

    F32 = mybir.dt.float32
    F16 = mybir.dt.float16
    AF = mybir.ActivationFunctionType
    g = bc * HIST
    r2 = g * HIST
    nbt = bc // 128              # b-chunks (2)
    nt1 = g // 128               # 128-row passes over one_hop / x_one_s (40)
    nt2 = r2 // 128
    nst = (g + ST_COLS - 1) // ST_COLS   # supertiles (10)
    nch = nt2 // CH              # two_hop DMA chunks (40)
    cps = TPS // CH              # chunks per supertile (4)
    xcs = ST_COLS // 128         # xos chunks per supertile (4)

    nc = bacc.Bacc("TRN2", target_bir_lowering=False, debug=False)

    def din(name, shape, dt=F16):
        return nc.dram_tensor(name, list(shape), dt, kind="ExternalInput").ap()

    two_hop = din("two_hop", (128, nt2 * FEAT))       # tiled [p, 128t+f]
    one_hop_n = din("one_hop_n", (128, nt1 * FEAT))   # tiled natural
    one_hop_t = din("one_hop_t", (FEAT, g))           # transposed
    self_t = din("self_t", (FEAT, bc))
    e1f = din("e1f", (128, nt1))          # exp(soft1 logit), flat-tiled [p,t]=v[128t+p]
    e1n = din("e1n", (bc, HIST))          # exp(soft1 logit), natural (for Z1)
    e2n = din("e2n", (bc, HIST * HIST))   # exp(soft2 logit), natural (for Z2)
    e2f = din("e2f", (128, nt2))          # exp(soft2 logit), flat-tiled
    w0t = din("w0t", (FEAT, HID))
    w2t = din("w2t", (FEAT, HID))
    w4t = din("w4t", (HID, OUT))
    w6t = din("w6t", (HID, OUT))
    b01 = din("b01", (1, HID))
    b46 = din("b46", (1, OUT))
    bdmask = din("bdmask", (128, 40))
    out_d = nc.dram_tensor("out", [bc, OUT], F32, kind="ExternalOutput").ap()

    with tile.TileContext(nc) as tc, ExitStack() as ctx:
        const = ctx.enter_context(tc.tile_pool(name="const", bufs=1))
        sbig = ctx.enter_context(tc.tile_pool(name="sbig", bufs=1))
        xpool = ctx.enter_context(tc.tile_pool(name="xp", bufs=6))
        bdpool = ctx.enter_context(tc.tile_pool(name="bdp", bufs=8))
        spool = ctx.enter_context(tc.tile_pool(name="sp", bufs=4))
        dpool = ctx.enter_context(tc.tile_pool(name="dram", bufs=1, space="DRAM"))
        p_agg = ctx.enter_context(tc.tile_pool(name="pagg", bufs=2, space="PSUM"))
        p_misc = ctx.enter_context(tc.tile_pool(name="pmisc", bufs=2, space="PSUM"))
        p_acc = ctx.enter_context(tc.tile_pool(name="pacc", bufs=1, space="PSUM"))

        # ---- priority DMAs (Activation engine queue): bd-weight inputs ----
        eflat_sb = const.tile([128, nt2], F16, tag="eflat")
        nc.scalar.dma_start(eflat_sb[:], e2f)
        mask_sb = const.tile([128, 40], F16, tag="mask")
        nc.scalar.dma_start(mask_sb[:], bdmask)
        e1f_sb = const.tile([128, nt1], F16, tag="e1f")
        nc.scalar.dma_start(e1f_sb[:], e1f)

        e1ts, e2ts = [], []
        for j in range(nbt):
            e1t = spool.tile([128, HIST], F16, tag=f"e1_{j}")
            nc.scalar.dma_start(e1t[:], e1n[128 * j:128 * (j + 1), :])
            e1ts.append(e1t)
            e2t = spool.tile([128, HIST * HIST], F16, tag=f"e2_{j}")
            nc.scalar.dma_start(e2t[:], e2n[128 * j:128 * (j + 1), :])
            e2ts.append(e2t)

        ones_row = const.tile([1, ST_COLS], F16, tag="ones")
        zeros_row = const.tile([1, ST_COLS], F16, tag="zeros")
        nc.vector.memset(ones_row[:], 1.0)
        nc.vector.memset(zeros_row[:], 0.0)

        # (body below may be repeated for the timing harness)
        for _rep in range(repeat):
          # ---- softmax normalizers (flat rows via tiny DRAM round-trip) ---
          # High priority: pins the reduce/recip chain to the front of the
          # DVE stream (ahead of the PE-paced bd builds) and the write kicks
          # to the front of the ACT stream.
          d_rz1 = dpool.tile([bc, 1], F16, tag="drz1")
          d_rz2 = dpool.tile([bc, HIST], F16, tag="drz2")
          with tc.high_priority():
              for j in range(nbt):
                  z1 = spool.tile([128, 1], F32, tag="z1")
                  nc.vector.reduce_sum(z1[:], e1ts[j][:], axis=mybir.AxisListType.X)
                  rz1 = spool.tile([128, 1], F16, tag="rz1")
                  with nc.allow_low_precision(reason="1/Z fp16: 5e-4, gate 2e-2"):
                      nc.vector.reciprocal(rz1[:], z1[:])
                  nc.scalar.dma_start(d_rz1[128 * j:128 * (j + 1), :], rz1[:])

                  z2 = spool.tile([128, HIST], F32, tag="z2")
                  nc.vector.reduce_sum(
                      z2[:],
                      e2ts[j][:].rearrange("p (h k) -> p h k", k=HIST),
                      axis=mybir.AxisListType.X,
                  )
                  rz2 = spool.tile([128, HIST], F16, tag="rz2")
                  with nc.allow_low_precision(reason="1/Z fp16: 5e-4, gate 2e-2"):
                      nc.vector.reciprocal(rz2[:], z2[:])
                  nc.scalar.dma_start(d_rz2[128 * j:128 * (j + 1), :], rz2[:])

              rz1row = const.tile([1, bc], F16, tag="rz1row")
              nc.gpsimd.dma_start(rz1row[:1, :], d_rz1[:].rearrange("a b -> (a b)"))
              rz2row = const.tile([1, g], F16, tag="rz2row")
              nc.gpsimd.dma_start(rz2row[:1, :], d_rz2[:].rearrange("a b -> (a b)"))

          # ---- remaining constants (Activation engine queue) --------------
          def cload(ap, shape, tag, dt=F16):
              t = const.tile(list(shape), dt, tag=tag)
              nc.scalar.dma_start(t[:], ap)
              return t

          w0t_sb = cload(w0t, (FEAT, HID), "w0t")
          w2t_sb = cload(w2t, (FEAT, HID), "w2t")
          w4t_sb = cload(w4t, (HID, OUT), "w4t")
          w6t_sb = cload(w6t, (HID, OUT), "w6t")
          b01_sb = cload(b01, (1, HID), "b01")
          b46_sb = cload(b46, (1, OUT), "b46")
          selft_sb = cload(self_t, (FEAT, bc), "selft")
          oht_sb = cload(one_hop_t, (FEAT, g), "oht")
          ohn_sb = cload(one_hop_n, (128, nt1 * FEAT), "ohn")

          # bd15 masks for the layer-2 (soft1) segment sums (DVE)
          bd15s = []
          for k in range(nt1 // 5):
              bd15 = const.tile([128, 40], F16, tag=f"bd15_{k}")
              nc.vector.tensor_mul(
                  bd15[:].rearrange("p (j m) -> p j m", m=8),
                  mask_sb[:].rearrange("p (j m) -> p j m", m=8),
                  e1f_sb[:, 5 * k:5 * k + 5].to_broadcast([128, 5, 8]),
              )
              bd15s.append(bd15)

          rz2rep_sb = sbig.tile([128, g], F32, tag="rz2rep")
          rz1rep_sb = sbig.tile([128, bc], F32, tag="rz1rep")
          agg2t_sb = sbig.tile([128, g], F16, tag="agg2t")
          xos_sb = sbig.tile([128, g], F16, tag="xos")
          yt_sb = sbig.tile([128, bc], F16, tag="yt")
          a1t_sb = sbig.tile([128, bc], F16, tag="a1t")
          py = p_acc.tile([128, bc], F32, tag="py")
          pa1 = p_acc.tile([128, bc], F32, tag="pa1")

          def emit_pre_tail():
              """PE/DVE setup work emitted alongside supertile 0's stream MMs:
              replicated 1/Z rows, and the py/pa1 zero-fills."""
              for s in range(nst):
                  cols = ST_COLS
                  rp = p_misc.tile([128, cols], F32, tag="misc")
                  nc.tensor.matmul(
                      rp[:], ones_row[:1, :128],
                      rz2row[:1, cols * s:cols * (s + 1)],
                      start=True, stop=True, skip_group_check=True,
                  )
                  nc.vector.tensor_copy(rz2rep_sb[:, cols * s:cols * (s + 1)], rp[:])
              rp = p_misc.tile([128, bc], F32, tag="misc")
              nc.tensor.matmul(rp[:], ones_row[:1, :128], rz1row[:1, :],
                               start=True, stop=True, skip_group_check=True)
              nc.vector.tensor_copy(rz1rep_sb[:], rp[:])
              nc.tensor.matmul(py[:], ones_row[:1, :128], zeros_row[:1, :bc],
                               start=True, stop=False, skip_group_check=True)
              nc.tensor.matmul(pa1[:], ones_row[:1, :128], zeros_row[:1, :bc],
                               start=True, stop=False, skip_group_check=True)

          def emit_tail(sp):
              """Phase-2 (x_one_s) + layer-2 masked matmuls for supertile sp;
              emitted while supertile sp+1 streams."""
              for ci in range(xcs):
                  c = xcs * sp + ci
                  p2 = p_misc.tile([128, HID], F32, tag="misc")
                  nc.tensor.matmul(
                      p2[:], ones_row[:1, :128], b01_sb[:1, :],
                      start=True, stop=False, skip_group_check=True,
                  )
                  nc.tensor.matmul(
                      p2[:], oht_sb[:, 128 * c:128 * (c + 1)], w0t_sb[:],
                      start=False, stop=False, skip_group_check=True,
                  )
                  nc.tensor.matmul(
                      p2[:], agg2t_sb[:, 128 * c:128 * (c + 1)], w2t_sb[:],
                      start=False, stop=True, skip_group_check=True,
                  )
                  nc.scalar.activation(
                      xos_sb[:, 128 * c:128 * (c + 1)], p2[:], AF.Relu)
              for ci in range(xcs):
                  t = xcs * sp + ci
                  w = _phase_width((128 * t) % 20)
                  bf = (128 * t) // 20
                  bd = bd15s[t // 5]
                  jj = t % 5
                  nc.tensor.matmul(
                      py[:, bf:bf + w], xos_sb[:, 128 * t:128 * (t + 1)],
                      bd[:, 8 * jj:8 * jj + w],
                      start=False, stop=(t == nt1 - 1), skip_group_check=True,
                  )
                  nc.tensor.matmul(
                      pa1[:, bf:bf + w], ohn_sb[:, 128 * t:128 * (t + 1)],
                      bd[:, 8 * jj:8 * jj + w],
                      start=False, stop=(t == nt1 - 1), skip_group_check=True,
                  )

          # ---- phase 1: agg2T[f, group], streamed --------------------------
          def emit_evict(s, pag_s):
              """Normalize + evict supertile s's PSUM accumulator (DVE)."""
              nc.vector.tensor_mul(
                  agg2t_sb[:, ST_COLS * s:ST_COLS * (s + 1)], pag_s[:],
                  rz2rep_sb[:, ST_COLS * s:ST_COLS * (s + 1)],
              )

          pag = None
          pending_evict = None
          for c in range(nch):
              xt = xpool.tile([128, CH * FEAT], F16, tag="x")
              if mode != "nodma":
                  nc.sync.dma_start(xt[:], two_hop[:, CH * FEAT * c:CH * FEAT * (c + 1)])
              # bd tiles for the whole chunk first: the DVE FIFO must hand the
              # PE its masks before blocking on eviction/copy work below.
              bds = []
              for h5 in range(CH // 5):
                  tg0 = CH * c + 5 * h5          # global tile idx of this 5-group
                  tl0 = tg0 % TPS                # tile idx within supertile
                  s = tg0 // TPS                 # supertile idx
                  if tl0 == 0:
                      pag = p_agg.tile([128, ST_COLS], F32, tag="agg")
                      nc.tensor.matmul(
                          pag[:], ones_row[:1, :128], zeros_row[:1, :ST_COLS],
                          start=True, stop=False, skip_group_check=True,
                      )
                  bd5 = bdpool.tile([128, 40], F16, tag="bd5")
                  nc.vector.tensor_mul(
                      bd5[:].rearrange("p (j m) -> p j m", m=8),
                      mask_sb[:].rearrange("p (j m) -> p j m", m=8),
                      eflat_sb[:, tg0:tg0 + 5].to_broadcast([128, 5, 8]),
                  )
                  bds.append(bd5)
              if mode == "dmaonly":
                  continue
              if c == cps:
                  # before the first eviction: DVE program order must have the
                  # rz2rep copies ahead of evict(s0)
                  emit_pre_tail()
              if pending_evict is not None:
                  emit_evict(*pending_evict)
                  pending_evict = None
              for h5 in range(CH // 5):
                  tg0 = CH * c + 5 * h5
                  tl0 = tg0 % TPS
                  s = tg0 // TPS
                  for j in range(5):
                      tl = tl0 + j
                      w = _phase_width((128 * tl) % 20)
                      gf = (128 * tl) // 20
                      nc.tensor.matmul(
                          pag[:, gf:gf + w],
                          xt[:, FEAT * (5 * h5 + j):FEAT * (5 * h5 + j + 1)],
                          bds[h5][:, 8 * j:8 * j + w],
                          start=False, stop=(tl == TPS - 1), skip_group_check=True,
                      )
                  if tl0 + 5 == TPS:
                      pending_evict = (s, pag)
              # interleave dependent tail work while the next chunks stream
              if c % cps == 1 and c > cps:
                  emit_tail(c // cps - 1)        # supertile finished a chunk ago
          if mode == "dmaonly":
              continue
          emit_evict(*pending_evict)
          pending_evict = None
          emit_tail(nst - 1)

          # ---- y/a1 evictions (1/Z1), x_s_one, final layer -----------------
          nc.vector.tensor_mul(yt_sb[:], py[:], rz1rep_sb[:])
          nc.vector.tensor_mul(a1t_sb[:], pa1[:], rz1rep_sb[:])

          pxs = p_acc.tile([128, bc], F32, tag="pxs")
          nc.tensor.matmul(pxs[:], b01_sb[:1, :], ones_row[:1, :bc],
                           start=True, stop=False, skip_group_check=True)
          nc.tensor.matmul(pxs[:], w0t_sb[:], selft_sb[:],
                           start=False, stop=False, skip_group_check=True)
          nc.tensor.matmul(pxs[:], w2t_sb[:], a1t_sb[:],
                           start=False, stop=True, skip_group_check=True)
          xst_sb = sbig.tile([128, bc], F16, tag="xst")
          nc.scalar.activation(xst_sb[:], pxs[:], AF.Relu)

          for j in range(nbt):
              po = p_misc.tile([128, OUT], F32, tag="misc")
              nc.tensor.matmul(po[:], ones_row[:1, :128], b46_sb[:1, :],
                               start=True, stop=False, skip_group_check=True)
              nc.tensor.matmul(po[:], xst_sb[:, 128 * j:128 * (j + 1)], w4t_sb[:],
                               start=False, stop=False, skip_group_check=True)
              nc.tensor.matmul(po[:], yt_sb[:, 128 * j:128 * (j + 1)], w6t_sb[:],
                               start=False, stop=True, skip_group_check=True)
              ot = spool.tile([128, OUT], F32, tag="ot")
              nc.scalar.copy(ot[:], po[:])
              nc.sync.dma_start(out_d[128 * j:128 * (j + 1), :], ot[:])

    nc.compile()
    return nc


def make_in_maps(inputs: dict, bc: int = BC, ncores: int = NCORES):
    """Host-side shard + auxiliary layout prep. Returns list of per-core dicts."""
    f32 = np.float32
    f16 = np.float16
    self_feat = np.asarray(inputs["self_feat"], f32)
    one_hop = np.asarray(inputs["one_hop_feat"], f32)
    two_hop = np.asarray(inputs["two_hop_feat"], f32)
    e_time = np.asarray(inputs["e_time"], f32)
    his_time = np.asarray(inputs["his_time"], f32)
    his_his = np.asarray(inputs["his_his_time"], f32)
    W0 = np.asarray(inputs["W0"], f32)
    b0 = np.asarray(inputs["b0"], f32)
    W2 = np.asarray(inputs["W2"], f32)
    b2 = np.asarray(inputs["b2"], f32)
    W4 = np.asarray(inputs["W4"], f32)
    b4 = np.asarray(inputs["b4"], f32)
    W6 = np.asarray(inputs["W6"], f32)
    b6 = np.asarray(inputs["b6"], f32)
    delta = float(np.asarray(inputs["delta"]).reshape(-1)[0])

    g = bc * HIST
    r2 = g * HIST

    def tiled(x):
        """[N*128, 128] -> [128, N*128] with row 128t+p at [p, 128t+f]."""
        n = x.shape[0] // 128
        return np.ascontiguousarray(
            x.reshape(n, 128, FEAT).transpose(1, 0, 2).reshape(128, n * FEAT)
        )

    C = np.ascontiguousarray
    shared = {
        "w0t": C(W0.T).astype(f16), "w2t": C(W2.T).astype(f16),
        "w4t": C(W4.T).astype(f16), "w6t": C(W6.T).astype(f16),
        "b01": (b0 + b2).reshape(1, HID).astype(f16),
        "b46": (b4 + b6).reshape(1, OUT).astype(f16),
        "bdmask": build_bdmask(),
    }
    maps = []
    for c in range(ncores):
        bs = slice(c * bc, (c + 1) * bc)
        oh = one_hop[c * g:(c + 1) * g]
        l1 = delta * (his_time[bs] - e_time[bs, None])    # [bc, H]
        l2 = delta * (his_his[bs] - his_time[bs, :, None])   # [bc, H, H]
        E1 = np.exp(l1).astype(f16)      # unnormalized softmax weights
        E2 = np.exp(l2).astype(f16)
        maps.append({
            "two_hop": tiled(two_hop[c * r2:(c + 1) * r2].astype(f16)),
            "one_hop_n": tiled(oh.astype(f16)),
            "one_hop_t": C(oh.T).astype(f16),
            "self_t": C(self_feat[bs].T).astype(f16),
            "e1f": C(E1.reshape(g // 128, 128).T),
            "e1n": C(E1.reshape(bc, HIST)),
            "e2n": C(E2.reshape(bc, HIST * HIST)),
            "e2f": C(E2.reshape(r2 // 128, 128).T),
            **shared,
        })
    return maps


def kernel(**inputs) -> np.ndarray:
    from concourse.bass_utils import run_bass_kernel_spmd

    nc = build_program(BC)
    in_maps = make_in_maps(inputs)
    res = run_bass_kernel_spmd(nc, in_maps, core_ids=list(range(NCORES)))
    return np.concatenate([res.results[c]["out"] for c in range(NCORES)], axis=0)
